# revision 7
# baseline (speedup 1.0000x reference)
"""Trainium2 Bass kernel for nn_AtBatCell: GRU recurrence over a shared state
table with gather/scatter-add per timestep.

Strategy: steps touching disjoint table rows are independent, so the T=8192
sequential scan collapses into waves (levels of the row-dependency DAG).
The device runs the first DW waves (87.5% of steps) as fully-packed batches
of 128-step GRU chunks; the small high-level tail (steps whose row chains
are 3+ deep) is finished on the host together with the delta assembly the
host already performs.

Device schedule (per core, SPMD-identical):
 - wave 1: all rows are first touches -> host-packed contiguous stream,
   plain DMA, no per-row descriptors. Steps whose rows are re-read by
   wave 2 are sorted first and their deltas scatter-added (SWDGE) into a
   small gather table G (~512 rows).
 - wave 2: rows come via dma_gather from G.
 - matmuls run in bf16 on the PE (f32 PSUM accumulate); H and r*h are
   PE-transposed in bf16.
 - deltas (dh) ship to DRAM contiguously; the host applies them and then
   computes the remaining tail waves directly (row chains are disjoint
   within a wave, so the tail is a few batched GEMMs).

Chunks are filled to exactly 128 steps by delaying "free" steps (steps no
later device step depends on) to later waves; component-based core
assignment keeps all touches of a row on one core."""
import os
import sys
for _p in ('/opt/trn_rl_repo', '/root/.axon_site/_ro/trn_rl_repo'):
    if os.path.isdir(_p) and _p not in sys.path:
        sys.path.insert(0, _p)

import collections
import numpy as np

SIT = 64
S = 256
S2 = 512
CHUNK = 128          # steps per compute chunk
SPARE = 128          # spare zero rows absorbing padding/dup scatters
BLOCK = 4            # chunks per gather/scatter/dh block
NCORES = 8
DW = 2               # device waves; later waves are finished on host
K_CAP = [5, 2]       # chunks per wave per core


def _schedule(b, p, n_rows_total):
    T = len(b)
    bl = b.astype(np.int64)
    pl = p.astype(np.int64)
    last = np.zeros(n_rows_total, np.int64)
    lev = np.empty(T, np.int64)
    for t in range(T):
        lv = max(last[bl[t]], last[pl[t]]) + 1
        lev[t] = lv
        last[bl[t]] = lv
        last[pl[t]] = lv

    rowtouch = collections.defaultdict(list)
    for t in range(T):
        rowtouch[bl[t]].append((t, 0))
        rowtouch[pl[t]].append((t, 1))
    nxt = np.full((T, 2), -1, np.int64)
    first = np.zeros((T, 2), bool)
    for r, lst in rowtouch.items():
        first[lst[0][0], lst[0][1]] = True
        for (t1, s1), (t2, _) in zip(lst, lst[1:]):
            nxt[t1, s1] = t2

    prov_dev = lev <= DW
    free = np.zeros(T, bool)
    for t in range(T):
        if not prov_dev[t]:
            continue
        free[t] = all(
            nxt[t, s] < 0 or not prov_dev[nxt[t, s]] for s in (0, 1))

    # union-find over provisional device steps
    parent = np.arange(T)

    def find(a):
        while parent[a] != a:
            parent[a] = parent[parent[a]]
            a = parent[a]
        return a

    for t in range(T):
        if not prov_dev[t]:
            continue
        for s in (0, 1):
            t2 = nxt[t, s]
            if t2 >= 0 and prov_dev[t2]:
                ra, rb = find(t), find(t2)
                if ra != rb:
                    parent[ra] = rb
    comp = collections.defaultdict(list)
    for t in range(T):
        if prov_dev[t]:
            comp[find(t)].append(t)
    comps = sorted(comp.values(), key=lambda v: (-len(v), v[0]))

    # balance components across cores on (per-level counts, total)
    targets = np.zeros(DW + 1)
    cvecs = []
    for cv in comps:
        v = np.zeros(DW + 1)
        for t in cv:
            v[lev[t] - 1] += 1
        v[DW] = len(cv)
        cvecs.append(v)
        targets += v
    targets = np.maximum(targets / NCORES, 1e-9)
    loads = np.zeros((NCORES, DW + 1))
    cassign = {}
    for cv, v in zip(comps, cvecs):
        cidx = int(np.argmin(((loads + v) / targets).max(axis=1)))
        loads[cidx] += v
        cassign[cv[0]] = cidx

    # per-core wave placement: nonfree at their level, free fill remaining
    # capacity (any wave >= their level), overflow goes to the host tail
    wave_steps = [[[] for _ in range(DW)] for _ in range(NCORES)]
    for cv in comps:
        c = cassign[cv[0]]
        for t in cv:
            if not free[t]:
                wave_steps[c][lev[t] - 1].append(t)
    for c in range(NCORES):
        for w in range(DW):
            assert len(wave_steps[c][w]) <= K_CAP[w] * CHUNK, \
                f"core {c} wave {w}: nonfree overflow"
    for cv in comps:
        c = cassign[cv[0]]
        for t in cv:
            if not free[t]:
                continue
            for w in range(int(lev[t]) - 1, DW):
                if len(wave_steps[c][w]) < K_CAP[w] * CHUNK:
                    wave_steps[c][w].append(t)
                    break
            # else: host tail

    dev_mask = np.zeros(T, bool)
    for c in range(NCORES):
        for w in range(DW):
            for t in wave_steps[c][w]:
                dev_mask[t] = True

    keep = np.zeros((T, 2), bool)
    for t in range(T):
        if dev_mask[t]:
            for s in (0, 1):
                keep[t, s] = nxt[t, s] >= 0 and dev_mask[nxt[t, s]]

    # keep-steps first within each wave (scatter prefix)
    for c in range(NCORES):
        for w in range(DW):
            wave_steps[c][w].sort(key=lambda t: (not keep[t].any(), t))

    host_steps = np.nonzero(~dev_mask)[0]

    # invariants
    for r, lst in rowtouch.items():
        seen_host = False
        for (t, s) in lst:
            if dev_mask[t]:
                assert not seen_host
            else:
                seen_host = True
    for c in range(NCORES):
        for t in wave_steps[c][0]:
            assert first[t].all(), "non-fresh slot in wave 1"

    return dict(lev=lev, nxt=nxt, first=first, keep=keep,
                wave_steps=wave_steps, host_steps=host_steps,
                dev_mask=dev_mask)


def _build_host_data(x, b, p, Wz, Wr, Wh, Uz, Ur, Uh, bz, br, bh, table0):
    import ml_dtypes
    bf16 = ml_dtypes.bfloat16
    N = table0.shape[0]
    b = b.astype(np.int64)
    p = p.astype(np.int64)
    sch = _schedule(b, p, N)
    keep, first = sch['keep'], sch['first']
    wave_steps = sch['wave_steps']

    wave_chunks = list(K_CAP)
    n_chunks = sum(wave_chunks)
    T_pad = n_chunks * CHUNK

    # scatter prefix: chunks holding keep-steps in wave 1..DW-1
    kc_wave = [0] * DW
    for w in range(DW - 1):
        mx = max(sum(1 for t in wave_steps[c][w] if keep[t].any())
                 for c in range(NCORES))
        kc_wave[w] = -(-mx // CHUNK)

    # blocks: (chunk_start, n_chunks, wave, all_fresh, keep_chunks, fresh_off)
    blocks = []
    fc = 0
    cl = 0
    for w, wc in enumerate(wave_chunks):
        for bstart in range(0, wc, BLOCK):
            nb = min(BLOCK, wc - bstart)
            af = (w == 0)
            kc = max(0, min(nb, kc_wave[w] - bstart))
            fo = -1
            if af:
                fo = fc
                fc += 2 * nb
            blocks.append((cl + bstart, nb, w, af, kc, fo))
        cl += wc
    fresh_cols = max(fc, 2)
    chunk_wave = np.repeat(np.arange(DW), wave_chunks)

    # per-core data
    per_core = []
    core_rows = []
    dup_any = False
    for c in range(NCORES):
        ob = np.full(T_pad, -1, np.int64)   # original row ids (host assembly)
        op = np.full(T_pad, -1, np.int64)
        x_c = np.zeros((T_pad, SIT), np.float32)
        bias_c = np.zeros(T_pad, np.float32)
        dm_c = np.zeros(T_pad, np.float32)
        st_c = np.full(T_pad, -1, np.int64)
        j0 = 0
        for w, wc in enumerate(wave_chunks):
            ts = wave_steps[c][w]
            sl = slice(j0, j0 + len(ts))
            tsa = np.asarray(ts, np.int64)
            if len(ts):
                st_c[sl] = tsa
                ob[sl] = b[tsa]
                op[sl] = p[tsa]
                x_c[sl] = x[tsa]
                bias_c[sl] = 1.0
            j0 += wc * CHUNK
        dup = (ob == op) & (ob >= 0)
        if dup.any():
            dup_any = True
            dm_c[dup] = 1.0
            op[dup] = -1          # p-side folded into b via dupmask
        # G rows: rows referenced by wave>=2 chunks
        gmask = np.zeros(T_pad, bool)
        gmask[K_CAP[0] * CHUNK:] = True
        rows = np.unique(np.concatenate([
            ob[gmask & (ob >= 0)], op[gmask & (op >= 0)]]))
        core_rows.append(rows)
        per_core.append(dict(ob=ob, op=op, x_c=x_c, bias_c=bias_c,
                             dm_c=dm_c, st=st_c))

    n_real_pc = max(len(r) for r in core_rows)
    n_rows_pc = n_real_pc + SPARE
    spare_ids = n_real_pc + np.arange(SPARE)
    spare_b = spare_ids[np.arange(CHUNK) % SPARE]
    spare_p = spare_ids[(np.arange(CHUNK) + 1) % SPARE]

    for c in range(NCORES):
        pc = per_core[c]
        rows = core_rows[c]
        remap = np.full(N, -1, np.int64)
        remap[rows] = np.arange(len(rows))
        ob, op, st = pc['ob'], pc['op'], pc['st']
        # device idx: G-local if row in G, else spare (pad/non-keep)
        bs = np.where(ob >= 0, remap[np.maximum(ob, 0)], -1)
        ps = np.where(op >= 0, remap[np.maximum(op, 0)], -1)
        bs = np.where(bs >= 0, bs, spare_b[np.arange(T_pad) % CHUNK])
        ps = np.where(ps >= 0, ps, spare_p[np.arange(T_pad) % CHUNK])
        # wave-2 chunks must address only real G rows
        g0 = K_CAP[0] * CHUNK
        assert (bs[g0:][st[g0:] >= 0] < n_real_pc).all()
        idx_il = np.stack([bs.reshape(-1, CHUNK), ps.reshape(-1, CHUNK)],
                          axis=1).reshape(-1).astype(np.int16)
        idx_rep = np.tile(idx_il.reshape(-1, 16).T, (8, 1)).copy()
        xT_c = np.zeros((SIT + 1, T_pad), np.float32)
        xT_c[:SIT] = pc['x_c'].T
        xT_c[SIT] = pc['bias_c']
        # fresh stream: wave-1 chunks, zeros for pads
        fresh_c = np.zeros((128, fresh_cols, S), np.float32)
        for (cs, nb, w, af, kc, fo) in blocks:
            if fo < 0:
                continue
            for q in range(nb):
                sl = slice((cs + q) * CHUNK, (cs + q + 1) * CHUNK)
                vb = pc['ob'][sl] >= 0
                vp = pc['op'][sl] >= 0
                fresh_c[vb, fo + 2 * q, :] = table0[pc['ob'][sl][vb]]
                fresh_c[vp, fo + 2 * q + 1, :] = table0[pc['op'][sl][vp]]
        tab_c = np.zeros((n_rows_pc, S), np.float32)
        tab_c[:len(rows)] = table0[rows]
        dmask_c = pc['dm_c'].reshape(n_chunks, CHUNK).T.copy()
        per_core[c] = dict(idx_rep=idx_rep, xT=xT_c.astype(bf16),
                           fresh_arr=fresh_c, dmask=dmask_c,
                           ob=ob, op=op, table_c=tab_c)

    WzT = np.concatenate([Wz.T, bz[None, :]], axis=0)
    WrT = np.concatenate([Wr.T, -br[None, :]], axis=0)
    WhT = np.concatenate([Wh.T, bh[None, :]], axis=0)

    def ut(U):
        return np.ascontiguousarray(U.T.reshape(4, 128, S2).transpose(1, 0, 2))

    hd = dict(
        WzT=WzT.astype(bf16), WrT=WrT.astype(bf16), WhT=WhT.astype(bf16),
        UzT=ut(Uz).astype(bf16), UrT=ut(Ur).astype(bf16),
        UhT=ut(Uh).astype(bf16),
        n_chunks=n_chunks, blocks=blocks, fresh_cols=fresh_cols,
        kc_wave=kc_wave, wave_chunks=wave_chunks, chunk_wave=chunk_wave,
        n_rows_c=n_rows_pc, T_pad=T_pad, dup_any=dup_any,
        per_core=per_core,
        host_steps=sch['host_steps'], lev=sch['lev'],
        x=x, b=b, p=p, Wz=Wz, Wr=Wr, Wh=Wh, Uz=Uz, Ur=Ur, Uh=Uh,
        bz=bz, br=br, bh=bh,
    )
    return hd


def _build_nc(hd):
    import concourse.bacc as bacc
    import concourse.mybir as mybir
    import concourse.tile as tile
    from concourse.masks import make_identity

    n_rows_c = hd['n_rows_c']
    n_chunks = hd['n_chunks']
    T_pad = hd['T_pad']
    blocks = hd['blocks']
    f32 = mybir.dt.float32
    bf16 = mybir.dt.bfloat16
    i16 = mybir.dt.int16

    nc = bacc.Bacc("TRN2", target_bir_lowering=False, debug=True)

    tab_in = nc.dram_tensor("table", (n_rows_c, S), f32, kind="ExternalInput")
    idx_in = nc.dram_tensor("idx", (128, 2 * T_pad // 16), i16, kind="ExternalInput")
    fresh_in = nc.dram_tensor("fresh", (128, hd['fresh_cols'], S), f32,
                              kind="ExternalInput")
    xT_in = nc.dram_tensor("xT", (SIT + 1, T_pad), bf16, kind="ExternalInput")
    WzT_in = nc.dram_tensor("WzT", (SIT + 1, S2), bf16, kind="ExternalInput")
    WrT_in = nc.dram_tensor("WrT", (SIT + 1, S2), bf16, kind="ExternalInput")
    WhT_in = nc.dram_tensor("WhT", (SIT + 1, S2), bf16, kind="ExternalInput")
    UzT_in = nc.dram_tensor("UzT", (128, 4, S2), bf16, kind="ExternalInput")
    UrT_in = nc.dram_tensor("UrT", (128, 4, S2), bf16, kind="ExternalInput")
    UhT_in = nc.dram_tensor("UhT", (128, 4, S2), bf16, kind="ExternalInput")
    dmask_in = nc.dram_tensor("dmask", (128, n_chunks), f32, kind="ExternalInput")

    dh_out = nc.dram_tensor("dh", (128, 2 * n_chunks, S), f32,
                            kind="ExternalOutput")
    tab_work = nc.dram_tensor("tabw", (n_rows_c, S), f32)  # internal scratch

    Sig = mybir.ActivationFunctionType.Sigmoid
    Tanh = mybir.ActivationFunctionType.Tanh

    wave_chunks = hd['wave_chunks']
    kc_wave = hd['kc_wave']
    chunk_wave = hd['chunk_wave']

    with tile.TileContext(nc) as tc:
        with tc.tile_pool(name="const", bufs=1) as cpool, \
             tc.tile_pool(name="gath", bufs=8) as gpool, \
             tc.tile_pool(name="dhb", bufs=4) as dhpool, \
             tc.tile_pool(name="work", bufs=4) as wpool, \
             tc.tile_pool(name="psA", bufs=2, space="PSUM") as psA, \
             tc.tile_pool(name="psZ", bufs=2, space="PSUM") as psZ, \
             tc.tile_pool(name="psR", bufs=2, space="PSUM") as psR, \
             tc.tile_pool(name="psM", bufs=2, space="PSUM") as psM:

            # ---- static loads (sync HWDGE), most-urgent first ----
            xT_sb = cpool.tile([SIT + 1, T_pad], bf16, tag="xT")
            nc.sync.dma_start(xT_sb[:], xT_in[:])
            w_sb = {}
            for nm, t in (("WzT", WzT_in), ("WrT", WrT_in)):
                w_sb[nm] = cpool.tile([SIT + 1, S2], bf16, tag=nm, name=nm + "_sb")
                nc.sync.dma_start(w_sb[nm][:], t[:])
            for nm, t in (("UzT", UzT_in), ("UrT", UrT_in)):
                w_sb[nm] = cpool.tile([128, 4, S2], bf16, tag=nm, name=nm + "_sb")
                nc.sync.dma_start(w_sb[nm][:], t[:])
            gtiles = {}

            def emit_fresh(c):
                g = gpool.tile([128, 2, S], f32, tag="hg", name=f"hg_{c}")
                nc.sync.dma_start(g[:], fresh_in[:, 2 * c:2 * c + 2, :])
                return g

            # first chunk's rows before the remaining weights
            gtiles[0] = emit_fresh(0)
            w_sb["WhT"] = cpool.tile([SIT + 1, S2], bf16, tag="WhT", name="WhT_sb")
            nc.sync.dma_start(w_sb["WhT"][:], WhT_in[:])
            w_sb["UhT"] = cpool.tile([128, 4, S2], bf16, tag="UhT", name="UhT_sb")
            nc.sync.dma_start(w_sb["UhT"][:], UhT_in[:])
            for c in range(1, wave_chunks[0]):
                gtiles[c] = emit_fresh(c)
            idx_sb = cpool.tile([128, 2 * T_pad // 16], i16, tag="idx")
            nc.sync.dma_start(idx_sb[:], idx_in[:])
            dmask_sb = cpool.tile([128, n_chunks], f32, tag="dmask")
            if hd['dup_any']:
                nc.sync.dma_start(dmask_sb[:], dmask_in[:])
            identb = cpool.tile([128, 128], bf16, tag="identb")
            make_identity(nc, identb[:])
            # gather-table init copy (SWDGE; only blocks wave-2 gathers)
            CP = 1024
            for r0 in range(0, n_rows_c, CP):
                nc.gpsimd.dma_start(tab_work[r0:min(r0 + CP, n_rows_c), :],
                                    tab_in[r0:min(r0 + CP, n_rows_c), :])

            def emit_gather(c):
                g = gpool.tile([128, 2, S], f32, tag="hg", name=f"hg_{c}")
                nc.gpsimd.dma_gather(
                    out_ap=g[:], in_ap=tab_work[:],
                    idxs_ap=idx_sb[:, 16 * c:16 * (c + 1)],
                    num_idxs=2 * CHUNK, num_idxs_reg=2 * CHUNK,
                    elem_size=S, queue_num=0,
                )
                return g

            cur_wave = 0
            for c in range(n_chunks):
                w = int(chunk_wave[c])
                if w != cur_wave:
                    cur_wave = w
                    for c2 in range(c, c + wave_chunks[w]):
                        gtiles[c2] = emit_gather(c2)

                g = gtiles.pop(c)
                hg2 = g[:].rearrange("p a b -> p (a b)")
                dhb = dhpool.tile([128, 2, S], f32, tag="dh", name=f"dh_{c}")

                # bf16 cast (gpsimd), then PE transpose of H in bf16
                hb = wpool.tile([128, S2], bf16, tag="hb")
                nc.gpsimd.tensor_copy(hb[:], hg2)
                tr_ps_f = psA.tile([128, 4, CHUNK], f32, tag="tr",
                                   name=f"trp_{c}")
                tr_ps = tr_ps_f[:].bitcast(bf16)
                ht_ps = tr_ps[:, :, 0:CHUNK]
                for k in range(4):
                    nc.tensor.transpose(
                        ht_ps[:, k, :], hb[:, CHUNK * k:CHUNK * (k + 1)],
                        identb[:])
                ht = wpool.tile([128, 4, CHUNK], bf16, tag="ht")
                nc.vector.tensor_copy(ht[:], ht_ps)

                xt_c = xT_sb[:, CHUNK * c:CHUNK * (c + 1)]

                zpre = psZ.tile([128, S2], f32, tag="zpre")
                rpre = psR.tile([128, S2], f32, tag="rpre")
                nc.tensor.matmul(zpre[:], xt_c, w_sb["WzT"][:],
                                 start=True, stop=False)
                nc.tensor.matmul(rpre[:], xt_c, w_sb["WrT"][:],
                                 start=True, stop=False)
                for k in range(4):
                    nc.tensor.matmul(zpre[:], ht[:, k, :], w_sb["UzT"][:, k, :],
                                     start=False, stop=(k == 3))
                    nc.tensor.matmul(rpre[:], ht[:, k, :], w_sb["UrT"][:, k, :],
                                     start=False, stop=(k == 3))

                zc = wpool.tile([128, S2], f32, tag="zc")
                r = wpool.tile([128, S2], f32, tag="r")
                nc.scalar.activation(zc[:], zpre[:], Sig, scale=-1.0)  # 1-z
                nc.scalar.activation(r[:], rpre[:], Sig)

                rh = wpool.tile([128, S2], bf16, tag="rh")
                nc.vector.tensor_mul(rh[:], r[:], hg2)
                rht_ps = tr_ps[:, :, CHUNK:2 * CHUNK]
                for k in range(4):
                    nc.tensor.transpose(
                        rht_ps[:, k, :], rh[:, CHUNK * k:CHUNK * (k + 1)],
                        identb[:])
                rht = wpool.tile([128, 4, CHUNK], bf16, tag="rht")
                nc.vector.tensor_copy(rht[:], rht_ps)

                mpre = psM.tile([128, S2], f32, tag="mpre")
                nc.tensor.matmul(mpre[:], xt_c, w_sb["WhT"][:],
                                 start=True, stop=False)
                for k in range(4):
                    nc.tensor.matmul(mpre[:], rht[:, k, :], w_sb["UhT"][:, k, :],
                                     start=False, stop=(k == 3))

                m = wpool.tile([128, S2], f32, tag="m")
                nc.scalar.activation(m[:], mpre[:], Tanh)

                # dh = (1-z)*(m-h)
                t1 = wpool.tile([128, S2], f32, tag="t1")
                nc.gpsimd.tensor_sub(t1[:], m[:], hg2)
                dh_view = dhb[:].rearrange("p a b -> p (a b)")
                nc.vector.tensor_mul(dh_view, zc[:], t1[:])
                if hd['dup_any']:
                    tm = wpool.tile([128, S], f32, tag="tm")
                    nc.vector.tensor_scalar_mul(
                        tm[:], dhb[:, 1, :], dmask_sb[:, c:c + 1])
                    nc.vector.tensor_add(
                        dhb[:, 0, :], dhb[:, 0, :], tm[:])

                # ship deltas to host (sync HWDGE)
                nc.sync.dma_start(dh_out[:, 2 * c:2 * c + 2, :], dhb[:])
                cw0 = c - int(np.sum(wave_chunks[:w]))
                if w < DW - 1 and cw0 < kc_wave[w]:
                    nc.gpsimd.dma_scatter_add(
                        tab_work[:], dhb[:],
                        idx_sb[:, 16 * c:16 * (c + 1)],
                        2 * CHUNK, 2 * CHUNK, S, queue_num=0,
                    )

    nc.compile()
    return nc


def _in_map(hd, core):
    pc = hd['per_core'][core]
    return {
        "table": pc['table_c'], "idx": pc['idx_rep'], "fresh": pc['fresh_arr'],
        "xT": pc['xT'],
        "WzT": hd['WzT'], "WrT": hd['WrT'], "WhT": hd['WhT'],
        "UzT": hd['UzT'], "UrT": hd['UrT'], "UhT": hd['UhT'],
        "dmask": pc['dmask'],
    }


def _run(hd, nc, trace=False):
    from concourse.bass_utils import run_bass_kernel_spmd
    return run_bass_kernel_spmd(nc, [_in_map(hd, c) for c in range(8)],
                                list(range(8)), trace=trace)


def _assemble(hd, dh_cores, table0):
    """Apply device deltas (rows never cross cores), then finish the tail
    waves on host (same-level steps never share a row -> batched GEMMs)."""
    n_chunks = hd['n_chunks']
    out = table0.astype(np.float32).copy()
    for cidx in range(8):
        dh = np.ascontiguousarray(dh_cores[cidx].transpose(1, 0, 2))
        dh = dh.reshape(n_chunks, 2, CHUNK, S).transpose(0, 2, 1, 3)
        dh = dh.reshape(hd['T_pad'] * 2, S)
        pc = hd['per_core'][cidx]
        rows = np.stack([pc['ob'], pc['op']], axis=1).reshape(-1)
        valid = rows >= 0
        np.add.at(out, rows[valid], dh[valid])

    hs = np.asarray(hd['host_steps'], np.int64)
    if len(hs):
        x, b, p = hd['x'], hd['b'], hd['p']
        Wz, Wr, Wh = hd['Wz'], hd['Wr'], hd['Wh']
        Uz, Ur, Uh = hd['Uz'], hd['Ur'], hd['Uh']
        bz, br, bh = hd['bz'], hd['br'], hd['bh']
        levs = hd['lev'][hs]
        for L in np.unique(levs):
            ts = hs[levs == L]
            H = np.concatenate([out[b[ts]], out[p[ts]]], axis=1)
            Z = 1 / (1 + np.exp(-(x[ts] @ Wz.T + H @ Uz.T + bz)))
            R = 1 / (1 + np.exp(-(x[ts] @ Wr.T + H @ Ur.T - br)))
            M = np.tanh(x[ts] @ Wh.T + (R * H) @ Uh.T + bh)
            dh = (1.0 - Z) * (M - H)
            np.add.at(out, b[ts], dh[:, :S])
            np.add.at(out, p[ts], dh[:, S:])
    return out


def kernel(**inputs):
    x = np.asarray(inputs['x'], dtype=np.float32)
    b = np.asarray(inputs['b'])
    p = np.asarray(inputs['p'])
    table0 = np.asarray(inputs['table0'], dtype=np.float32)

    hd = _build_host_data(
        x, b, p,
        np.asarray(inputs['Wz'], np.float32), np.asarray(inputs['Wr'], np.float32),
        np.asarray(inputs['Wh'], np.float32), np.asarray(inputs['Uz'], np.float32),
        np.asarray(inputs['Ur'], np.float32), np.asarray(inputs['Uh'], np.float32),
        np.asarray(inputs['bz'], np.float32), np.asarray(inputs['br'], np.float32),
        np.asarray(inputs['bh'], np.float32), table0)

    nc = _build_nc(hd)
    res = _run(hd, nc)
    dh_cores = [np.asarray(res.results[c]["dh"], np.float32) for c in range(8)]
    return _assemble(hd, dh_cores, table0)


if __name__ == "__main__":
    d = np.load('/tmp/ref_inputs.npz')
    inputs = {k: d[k] for k in d.files}
    got = kernel(**inputs)
    exp = np.load('/tmp/ref_out_np.npy')
    err = np.abs(got - exp).max()
    print("abs err:", err, "rel:", err / np.abs(exp).max())


# revision 10
# speedup vs baseline: 1.3858x; 1.3858x over previous
"""Trainium2 Bass kernel for nn_AtBatCell: GRU recurrence over a shared state
table with gather/scatter-add per timestep.

Strategy: steps touching disjoint table rows are independent, so the T=8192
sequential scan collapses into waves (levels of the row-dependency DAG).
The device runs the first DW waves (87.5% of steps) as fully-packed batches
of 128-step GRU chunks; the small high-level tail (steps whose row chains
are 3+ deep) is finished on the host together with the delta assembly the
host already performs.

Device schedule (per core, SPMD-identical):
 - wave 1: all rows are first touches -> host-packed contiguous stream,
   plain DMA, no per-row descriptors. Steps whose rows are re-read by
   wave 2 are sorted first and their deltas scatter-added (SWDGE) into a
   small gather table G (~512 rows).
 - wave 2: rows come via dma_gather from G.
 - matmuls run in bf16 on the PE (f32 PSUM accumulate); H and r*h are
   PE-transposed in bf16.
 - deltas (dh) ship to DRAM contiguously; the host applies them and then
   computes the remaining tail waves directly (row chains are disjoint
   within a wave, so the tail is a few batched GEMMs).

Chunks are filled to exactly 128 steps by delaying "free" steps (steps no
later device step depends on) to later waves; component-based core
assignment keeps all touches of a row on one core."""
import os
import sys
for _p in ('/opt/trn_rl_repo', '/root/.axon_site/_ro/trn_rl_repo'):
    if os.path.isdir(_p) and _p not in sys.path:
        sys.path.insert(0, _p)

import collections
import numpy as np

SIT = 64
S = 256
S2 = 512
CHUNK = 128          # steps per compute chunk
SPARE = 128          # spare zero rows absorbing padding/dup scatters
BLOCK = 4            # chunks per gather/scatter/dh block
NCORES = 8
DW = 2               # device waves; later waves are finished on host
K_CAP = [5, 2]       # chunks per wave per core


def _schedule(b, p, n_rows_total):
    T = len(b)
    bl = b.astype(np.int64)
    pl = p.astype(np.int64)
    last = np.zeros(n_rows_total, np.int64)
    lev = np.empty(T, np.int64)
    for t in range(T):
        lv = max(last[bl[t]], last[pl[t]]) + 1
        lev[t] = lv
        last[bl[t]] = lv
        last[pl[t]] = lv

    rowtouch = collections.defaultdict(list)
    for t in range(T):
        rowtouch[bl[t]].append((t, 0))
        rowtouch[pl[t]].append((t, 1))
    nxt = np.full((T, 2), -1, np.int64)
    first = np.zeros((T, 2), bool)
    for r, lst in rowtouch.items():
        first[lst[0][0], lst[0][1]] = True
        for (t1, s1), (t2, _) in zip(lst, lst[1:]):
            nxt[t1, s1] = t2

    prov_dev = lev <= DW
    free = np.zeros(T, bool)
    for t in range(T):
        if not prov_dev[t]:
            continue
        free[t] = all(
            nxt[t, s] < 0 or not prov_dev[nxt[t, s]] for s in (0, 1))

    # union-find over provisional device steps
    parent = np.arange(T)

    def find(a):
        while parent[a] != a:
            parent[a] = parent[parent[a]]
            a = parent[a]
        return a

    for t in range(T):
        if not prov_dev[t]:
            continue
        for s in (0, 1):
            t2 = nxt[t, s]
            if t2 >= 0 and prov_dev[t2]:
                ra, rb = find(t), find(t2)
                if ra != rb:
                    parent[ra] = rb
    comp = collections.defaultdict(list)
    for t in range(T):
        if prov_dev[t]:
            comp[find(t)].append(t)
    comps = sorted(comp.values(), key=lambda v: (-len(v), v[0]))

    # balance components across cores on (per-level counts, total)
    targets = np.zeros(DW + 1)
    cvecs = []
    for cv in comps:
        v = np.zeros(DW + 1)
        for t in cv:
            v[lev[t] - 1] += 1
        v[DW] = len(cv)
        cvecs.append(v)
        targets += v
    targets = np.maximum(targets / NCORES, 1e-9)
    loads = np.zeros((NCORES, DW + 1))
    cassign = {}
    for cv, v in zip(comps, cvecs):
        cidx = int(np.argmin(((loads + v) / targets).max(axis=1)))
        loads[cidx] += v
        cassign[cv[0]] = cidx

    # per-core wave placement: nonfree at their level, free fill remaining
    # capacity (any wave >= their level), overflow goes to the host tail
    wave_steps = [[[] for _ in range(DW)] for _ in range(NCORES)]
    for cv in comps:
        c = cassign[cv[0]]
        for t in cv:
            if not free[t]:
                wave_steps[c][lev[t] - 1].append(t)
    for c in range(NCORES):
        for w in range(DW):
            assert len(wave_steps[c][w]) <= K_CAP[w] * CHUNK, \
                f"core {c} wave {w}: nonfree overflow"
    for cv in comps:
        c = cassign[cv[0]]
        for t in cv:
            if not free[t]:
                continue
            for w in range(int(lev[t]) - 1, DW):
                if len(wave_steps[c][w]) < K_CAP[w] * CHUNK:
                    wave_steps[c][w].append(t)
                    break
            # else: host tail

    dev_mask = np.zeros(T, bool)
    for c in range(NCORES):
        for w in range(DW):
            for t in wave_steps[c][w]:
                dev_mask[t] = True

    keep = np.zeros((T, 2), bool)
    for t in range(T):
        if dev_mask[t]:
            for s in (0, 1):
                keep[t, s] = nxt[t, s] >= 0 and dev_mask[nxt[t, s]]

    # keep-steps first within each wave (scatter prefix)
    for c in range(NCORES):
        for w in range(DW):
            wave_steps[c][w].sort(key=lambda t: (not keep[t].any(), t))

    host_steps = np.nonzero(~dev_mask)[0]

    # invariants
    for r, lst in rowtouch.items():
        seen_host = False
        for (t, s) in lst:
            if dev_mask[t]:
                assert not seen_host
            else:
                seen_host = True
    for c in range(NCORES):
        for t in wave_steps[c][0]:
            assert first[t].all(), "non-fresh slot in wave 1"

    return dict(lev=lev, nxt=nxt, first=first, keep=keep,
                wave_steps=wave_steps, host_steps=host_steps,
                dev_mask=dev_mask)


def _build_host_data(x, b, p, Wz, Wr, Wh, Uz, Ur, Uh, bz, br, bh, table0):
    import ml_dtypes
    bf16 = ml_dtypes.bfloat16
    N = table0.shape[0]
    b = b.astype(np.int64)
    p = p.astype(np.int64)
    sch = _schedule(b, p, N)
    keep, first = sch['keep'], sch['first']
    wave_steps = sch['wave_steps']

    wave_chunks = list(K_CAP)
    n_chunks = sum(wave_chunks)
    T_pad = n_chunks * CHUNK

    # scatter prefix: chunks holding keep-steps in wave 1..DW-1
    kc_wave = [0] * DW
    for w in range(DW - 1):
        mx = max(sum(1 for t in wave_steps[c][w] if keep[t].any())
                 for c in range(NCORES))
        kc_wave[w] = -(-mx // CHUNK)

    # blocks: (chunk_start, n_chunks, wave, all_fresh, keep_chunks, fresh_off)
    blocks = []
    fc = 0
    cl = 0
    for w, wc in enumerate(wave_chunks):
        for bstart in range(0, wc, BLOCK):
            nb = min(BLOCK, wc - bstart)
            af = (w == 0)
            kc = max(0, min(nb, kc_wave[w] - bstart))
            fo = -1
            if af:
                fo = fc
                fc += 2 * nb
            blocks.append((cl + bstart, nb, w, af, kc, fo))
        cl += wc
    fresh_cols = max(fc, 2)
    chunk_wave = np.repeat(np.arange(DW), wave_chunks)

    # per-core data
    per_core = []
    core_rows = []
    dup_any = False
    for c in range(NCORES):
        ob = np.full(T_pad, -1, np.int64)   # original row ids (host assembly)
        op = np.full(T_pad, -1, np.int64)
        x_c = np.zeros((T_pad, SIT), np.float32)
        bias_c = np.zeros(T_pad, np.float32)
        dm_c = np.zeros(T_pad, np.float32)
        st_c = np.full(T_pad, -1, np.int64)
        j0 = 0
        for w, wc in enumerate(wave_chunks):
            ts = wave_steps[c][w]
            sl = slice(j0, j0 + len(ts))
            tsa = np.asarray(ts, np.int64)
            if len(ts):
                st_c[sl] = tsa
                ob[sl] = b[tsa]
                op[sl] = p[tsa]
                x_c[sl] = x[tsa]
                bias_c[sl] = 1.0
            j0 += wc * CHUNK
        dup = (ob == op) & (ob >= 0)
        if dup.any():
            dup_any = True
            dm_c[dup] = 1.0
            op[dup] = -1          # p-side folded into b via dupmask
        # G rows: rows referenced by wave>=2 chunks
        gmask = np.zeros(T_pad, bool)
        gmask[K_CAP[0] * CHUNK:] = True
        rows = np.unique(np.concatenate([
            ob[gmask & (ob >= 0)], op[gmask & (op >= 0)]]))
        core_rows.append(rows)
        per_core.append(dict(ob=ob, op=op, x_c=x_c, bias_c=bias_c,
                             dm_c=dm_c, st=st_c))

    n_real_pc = max(len(r) for r in core_rows)
    n_rows_pc = n_real_pc + SPARE
    spare_ids = n_real_pc + np.arange(SPARE)
    spare_b = spare_ids[np.arange(CHUNK) % SPARE]
    spare_p = spare_ids[(np.arange(CHUNK) + 1) % SPARE]

    for c in range(NCORES):
        pc = per_core[c]
        rows = core_rows[c]
        remap = np.full(N, -1, np.int64)
        remap[rows] = np.arange(len(rows))
        ob, op, st = pc['ob'], pc['op'], pc['st']
        # device idx: G-local if row in G, else spare (pad/non-keep)
        bs = np.where(ob >= 0, remap[np.maximum(ob, 0)], -1)
        ps = np.where(op >= 0, remap[np.maximum(op, 0)], -1)
        bs = np.where(bs >= 0, bs, spare_b[np.arange(T_pad) % CHUNK])
        ps = np.where(ps >= 0, ps, spare_p[np.arange(T_pad) % CHUNK])
        # wave-2 chunks must address only real G rows
        g0 = K_CAP[0] * CHUNK
        assert (bs[g0:][st[g0:] >= 0] < n_real_pc).all()
        idx_il = np.stack([bs.reshape(-1, CHUNK), ps.reshape(-1, CHUNK)],
                          axis=1).reshape(-1).astype(np.int16)
        idx_rep = np.tile(idx_il.reshape(-1, 16).T, (8, 1)).copy()
        xT_c = np.zeros((SIT + 1, T_pad), np.float32)
        xT_c[:SIT] = pc['x_c'].T
        xT_c[SIT] = pc['bias_c']
        # fresh stream: wave-1 chunks, zeros for pads; shipped bf16 both in
        # natural [step, 2S] layout and pre-transposed [state, step] layout
        # (the transposed copy feeds the PE stationary directly)
        fresh_c = np.zeros((128, fresh_cols, S), np.float32)
        for (cs, nb, w, af, kc, fo) in blocks:
            if fo < 0:
                continue
            for q in range(nb):
                sl = slice((cs + q) * CHUNK, (cs + q + 1) * CHUNK)
                vb = pc['ob'][sl] >= 0
                vp = pc['op'][sl] >= 0
                fresh_c[vb, fo + 2 * q, :] = table0[pc['ob'][sl][vb]]
                fresh_c[vp, fo + 2 * q + 1, :] = table0[pc['op'][sl][vp]]
        fresh_b16 = fresh_c.astype(bf16)
        k1 = wave_chunks[0]
        freshT = np.zeros((128, k1, 4, CHUNK), bf16)
        for q in range(k1):
            hcat = np.concatenate([fresh_b16[:, 2 * q, :],
                                   fresh_b16[:, 2 * q + 1, :]], axis=1)
            for k in range(4):
                freshT[:, q, k, :] = hcat[:, CHUNK * k:CHUNK * (k + 1)].T
        tab_c = np.zeros((n_rows_pc, S), bf16)
        tab_c[:len(rows)] = table0[rows].astype(bf16)
        dmask_c = pc['dm_c'].reshape(n_chunks, CHUNK).T.copy()
        per_core[c] = dict(idx_rep=idx_rep, xT=xT_c.astype(bf16),
                           fresh_arr=fresh_b16, freshT=freshT, dmask=dmask_c,
                           ob=ob, op=op, table_c=tab_c)

    WzT = np.concatenate([Wz.T, bz[None, :]], axis=0)
    WrT = np.concatenate([Wr.T, -br[None, :]], axis=0)
    WhT = np.concatenate([Wh.T, bh[None, :]], axis=0)

    def ut(U):
        return np.ascontiguousarray(U.T.reshape(4, 128, S2).transpose(1, 0, 2))

    hd = dict(
        WzT=WzT.astype(bf16), WrT=WrT.astype(bf16), WhT=WhT.astype(bf16),
        UzT=ut(Uz).astype(bf16), UrT=ut(Ur).astype(bf16),
        UhT=ut(Uh).astype(bf16),
        n_chunks=n_chunks, blocks=blocks, fresh_cols=fresh_cols,
        kc_wave=kc_wave, wave_chunks=wave_chunks, chunk_wave=chunk_wave,
        n_rows_c=n_rows_pc, T_pad=T_pad, dup_any=dup_any,
        per_core=per_core,
        host_steps=sch['host_steps'], lev=sch['lev'],
        x=x, b=b, p=p, Wz=Wz, Wr=Wr, Wh=Wh, Uz=Uz, Ur=Ur, Uh=Uh,
        bz=bz, br=br, bh=bh,
    )
    return hd


def _build_nc(hd):
    import concourse.bacc as bacc
    import concourse.mybir as mybir
    import concourse.tile as tile
    from concourse.masks import make_identity

    n_rows_c = hd['n_rows_c']
    n_chunks = hd['n_chunks']
    T_pad = hd['T_pad']
    blocks = hd['blocks']
    f32 = mybir.dt.float32
    bf16 = mybir.dt.bfloat16
    i16 = mybir.dt.int16

    nc = bacc.Bacc("TRN2", target_bir_lowering=False, debug=True)

    tab_in = nc.dram_tensor("table", (n_rows_c, S), bf16, kind="ExternalInput")
    idx_in = nc.dram_tensor("idx", (128, 2 * T_pad // 16), i16, kind="ExternalInput")
    fresh_in = nc.dram_tensor("fresh", (128, hd['fresh_cols'], S), bf16,
                              kind="ExternalInput")
    freshT_in = nc.dram_tensor("freshT", (128, hd['wave_chunks'][0], 4, CHUNK),
                               bf16, kind="ExternalInput")
    xT_in = nc.dram_tensor("xT", (SIT + 1, T_pad), bf16, kind="ExternalInput")
    WzT_in = nc.dram_tensor("WzT", (SIT + 1, S2), bf16, kind="ExternalInput")
    WrT_in = nc.dram_tensor("WrT", (SIT + 1, S2), bf16, kind="ExternalInput")
    WhT_in = nc.dram_tensor("WhT", (SIT + 1, S2), bf16, kind="ExternalInput")
    UzT_in = nc.dram_tensor("UzT", (128, 4, S2), bf16, kind="ExternalInput")
    UrT_in = nc.dram_tensor("UrT", (128, 4, S2), bf16, kind="ExternalInput")
    UhT_in = nc.dram_tensor("UhT", (128, 4, S2), bf16, kind="ExternalInput")
    dmask_in = nc.dram_tensor("dmask", (128, n_chunks), f32, kind="ExternalInput")

    dh_out = nc.dram_tensor("dh", (128, 2 * n_chunks, S), bf16,
                            kind="ExternalOutput")
    tab_work = nc.dram_tensor("tabw", (n_rows_c, S), bf16)  # internal scratch

    Sig = mybir.ActivationFunctionType.Sigmoid
    Tanh = mybir.ActivationFunctionType.Tanh

    wave_chunks = hd['wave_chunks']
    kc_wave = hd['kc_wave']
    chunk_wave = hd['chunk_wave']

    with tile.TileContext(nc) as tc:
        with tc.tile_pool(name="const", bufs=1) as cpool, \
             tc.tile_pool(name="gath", bufs=8) as gpool, \
             tc.tile_pool(name="dhb", bufs=4) as dhpool, \
             tc.tile_pool(name="work", bufs=4) as wpool, \
             tc.tile_pool(name="psA", bufs=2, space="PSUM") as psA, \
             tc.tile_pool(name="psZ", bufs=2, space="PSUM") as psZ, \
             tc.tile_pool(name="psR", bufs=2, space="PSUM") as psR, \
             tc.tile_pool(name="psM", bufs=2, space="PSUM") as psM:

            # ---- static loads (sync HWDGE), most-urgent first ----
            xT_sb = cpool.tile([SIT + 1, T_pad], bf16, tag="xT")
            nc.sync.dma_start(xT_sb[:], xT_in[:])
            w_sb = {}
            for nm, t in (("WzT", WzT_in), ("WrT", WrT_in)):
                w_sb[nm] = cpool.tile([SIT + 1, S2], bf16, tag=nm, name=nm + "_sb")
                nc.sync.dma_start(w_sb[nm][:], t[:])
            for nm, t in (("UzT", UzT_in), ("UrT", UrT_in)):
                w_sb[nm] = cpool.tile([128, 4, S2], bf16, tag=nm, name=nm + "_sb")
                nc.sync.dma_start(w_sb[nm][:], t[:])
            gtiles = {}
            httiles = {}

            def emit_fresh(c):
                g = gpool.tile([128, 2, S], bf16, tag="hg", name=f"hg_{c}")
                nc.sync.dma_start(g[:], fresh_in[:, 2 * c:2 * c + 2, :])
                ht = gpool.tile([128, 4, CHUNK], bf16, tag="hT", name=f"hT_{c}")
                nc.sync.dma_start(ht[:], freshT_in[:, c, :, :])
                return g, ht

            # first chunk's rows before the remaining weights
            gtiles[0], httiles[0] = emit_fresh(0)
            w_sb["WhT"] = cpool.tile([SIT + 1, S2], bf16, tag="WhT", name="WhT_sb")
            nc.sync.dma_start(w_sb["WhT"][:], WhT_in[:])
            w_sb["UhT"] = cpool.tile([128, 4, S2], bf16, tag="UhT", name="UhT_sb")
            nc.sync.dma_start(w_sb["UhT"][:], UhT_in[:])
            # gather-table init copy (sync ring D2D; only blocks wave-2)
            CP = 256
            for r0 in range(0, n_rows_c, CP):
                nc.sync.dma_start(tab_work[r0:min(r0 + CP, n_rows_c), :],
                                  tab_in[r0:min(r0 + CP, n_rows_c), :])
            for c in range(1, wave_chunks[0]):
                gtiles[c], httiles[c] = emit_fresh(c)
            idx_sb = cpool.tile([128, 2 * T_pad // 16], i16, tag="idx")
            nc.sync.dma_start(idx_sb[:], idx_in[:])
            dmask_sb = cpool.tile([128, n_chunks], f32, tag="dmask")
            if hd['dup_any']:
                nc.sync.dma_start(dmask_sb[:], dmask_in[:])
            identb = cpool.tile([128, 128], bf16, tag="identb")
            make_identity(nc, identb[:])

            def emit_gather(c):
                g = gpool.tile([128, 2, S], bf16, tag="hg", name=f"hg_{c}")
                nc.gpsimd.dma_gather(
                    out_ap=g[:], in_ap=tab_work[:],
                    idxs_ap=idx_sb[:, 16 * c:16 * (c + 1)],
                    num_idxs=2 * CHUNK, num_idxs_reg=2 * CHUNK,
                    elem_size=S, queue_num=0,
                )
                return g

            cur_wave = 0
            for c in range(n_chunks):
                w = int(chunk_wave[c])
                if w != cur_wave:
                    cur_wave = w
                    for c2 in range(c, c + wave_chunks[w]):
                        gtiles[c2] = emit_gather(c2)

                g = gtiles.pop(c)
                hg2 = g[:].rearrange("p a b -> p (a b)")
                dhb = dhpool.tile([128, 2, S], bf16, tag="dh", name=f"dh_{c}")

                tr_ps_f = psA.tile([128, 4, CHUNK], f32, tag="tr",
                                   name=f"trp_{c}")
                tr_ps = tr_ps_f[:].bitcast(bf16)
                if c in httiles:
                    ht = httiles.pop(c)      # host-packed transposed stream
                else:
                    # PE transpose of the gathered (bf16) rows
                    ht_ps = tr_ps[:, :, 0:CHUNK]
                    for k in range(4):
                        nc.tensor.transpose(
                            ht_ps[:, k, :], hg2[:, CHUNK * k:CHUNK * (k + 1)],
                            identb[:])
                    ht = wpool.tile([128, 4, CHUNK], bf16, tag="ht")
                    nc.vector.tensor_copy(ht[:], ht_ps)

                xt_c = xT_sb[:, CHUNK * c:CHUNK * (c + 1)]

                zpre = psZ.tile([128, S2], f32, tag="zpre")
                rpre = psR.tile([128, S2], f32, tag="rpre")
                nc.tensor.matmul(zpre[:], xt_c, w_sb["WzT"][:],
                                 start=True, stop=False)
                nc.tensor.matmul(rpre[:], xt_c, w_sb["WrT"][:],
                                 start=True, stop=False)
                for k in range(4):
                    nc.tensor.matmul(zpre[:], ht[:, k, :], w_sb["UzT"][:, k, :],
                                     start=False, stop=(k == 3))
                    nc.tensor.matmul(rpre[:], ht[:, k, :], w_sb["UrT"][:, k, :],
                                     start=False, stop=(k == 3))

                zc = wpool.tile([128, S2], f32, tag="zc")
                r = wpool.tile([128, S2], f32, tag="r")
                nc.scalar.activation(zc[:], zpre[:], Sig, scale=-1.0)  # 1-z
                nc.scalar.activation(r[:], rpre[:], Sig)

                rh = wpool.tile([128, S2], bf16, tag="rh")
                nc.vector.tensor_mul(rh[:], r[:], hg2)
                rht_ps = tr_ps[:, :, CHUNK:2 * CHUNK]
                for k in range(4):
                    nc.tensor.transpose(
                        rht_ps[:, k, :], rh[:, CHUNK * k:CHUNK * (k + 1)],
                        identb[:])
                rht = wpool.tile([128, 4, CHUNK], bf16, tag="rht")
                nc.vector.tensor_copy(rht[:], rht_ps)

                mpre = psM.tile([128, S2], f32, tag="mpre")
                nc.tensor.matmul(mpre[:], xt_c, w_sb["WhT"][:],
                                 start=True, stop=False)
                for k in range(4):
                    nc.tensor.matmul(mpre[:], rht[:, k, :], w_sb["UhT"][:, k, :],
                                     start=False, stop=(k == 3))

                m = wpool.tile([128, S2], f32, tag="m")
                nc.scalar.activation(m[:], mpre[:], Tanh)

                # dh = (1-z)*(m-h)
                t1 = wpool.tile([128, S2], f32, tag="t1")
                nc.vector.tensor_sub(t1[:], m[:], hg2)
                dh_view = dhb[:].rearrange("p a b -> p (a b)")
                nc.vector.tensor_mul(dh_view, zc[:], t1[:])
                if hd['dup_any']:
                    tm = wpool.tile([128, S], bf16, tag="tm")
                    nc.vector.tensor_scalar_mul(
                        tm[:], dhb[:, 1, :], dmask_sb[:, c:c + 1])
                    nc.vector.tensor_add(
                        dhb[:, 0, :], dhb[:, 0, :], tm[:])

                # ship deltas to host (sync HWDGE)
                nc.sync.dma_start(dh_out[:, 2 * c:2 * c + 2, :], dhb[:])
                cw0 = c - int(np.sum(wave_chunks[:w]))
                if w < DW - 1 and cw0 < kc_wave[w]:
                    nc.gpsimd.dma_scatter_add(
                        tab_work[:], dhb[:],
                        idx_sb[:, 16 * c:16 * (c + 1)],
                        2 * CHUNK, 2 * CHUNK, S, queue_num=0,
                    )

    nc.compile()
    return nc


def _in_map(hd, core):
    pc = hd['per_core'][core]
    return {
        "table": pc['table_c'], "idx": pc['idx_rep'], "fresh": pc['fresh_arr'],
        "freshT": pc['freshT'], "xT": pc['xT'],
        "WzT": hd['WzT'], "WrT": hd['WrT'], "WhT": hd['WhT'],
        "UzT": hd['UzT'], "UrT": hd['UrT'], "UhT": hd['UhT'],
        "dmask": pc['dmask'],
    }


def _run(hd, nc, trace=False):
    from concourse.bass_utils import run_bass_kernel_spmd
    return run_bass_kernel_spmd(nc, [_in_map(hd, c) for c in range(8)],
                                list(range(8)), trace=trace)


def _assemble(hd, dh_cores, table0):
    """Apply device deltas (rows never cross cores), then finish the tail
    waves on host (same-level steps never share a row -> batched GEMMs)."""
    n_chunks = hd['n_chunks']
    out = table0.astype(np.float32).copy()
    for cidx in range(8):
        dh = np.ascontiguousarray(dh_cores[cidx].transpose(1, 0, 2))
        dh = dh.reshape(n_chunks, 2, CHUNK, S).transpose(0, 2, 1, 3)
        dh = dh.reshape(hd['T_pad'] * 2, S)
        pc = hd['per_core'][cidx]
        rows = np.stack([pc['ob'], pc['op']], axis=1).reshape(-1)
        valid = rows >= 0
        np.add.at(out, rows[valid], dh[valid])

    hs = np.asarray(hd['host_steps'], np.int64)
    if len(hs):
        x, b, p = hd['x'], hd['b'], hd['p']
        Wz, Wr, Wh = hd['Wz'], hd['Wr'], hd['Wh']
        Uz, Ur, Uh = hd['Uz'], hd['Ur'], hd['Uh']
        bz, br, bh = hd['bz'], hd['br'], hd['bh']
        levs = hd['lev'][hs]
        for L in np.unique(levs):
            ts = hs[levs == L]
            H = np.concatenate([out[b[ts]], out[p[ts]]], axis=1)
            Z = 1 / (1 + np.exp(-(x[ts] @ Wz.T + H @ Uz.T + bz)))
            R = 1 / (1 + np.exp(-(x[ts] @ Wr.T + H @ Ur.T - br)))
            M = np.tanh(x[ts] @ Wh.T + (R * H) @ Uh.T + bh)
            dh = (1.0 - Z) * (M - H)
            np.add.at(out, b[ts], dh[:, :S])
            np.add.at(out, p[ts], dh[:, S:])
    return out


def kernel(**inputs):
    x = np.asarray(inputs['x'], dtype=np.float32)
    b = np.asarray(inputs['b'])
    p = np.asarray(inputs['p'])
    table0 = np.asarray(inputs['table0'], dtype=np.float32)

    hd = _build_host_data(
        x, b, p,
        np.asarray(inputs['Wz'], np.float32), np.asarray(inputs['Wr'], np.float32),
        np.asarray(inputs['Wh'], np.float32), np.asarray(inputs['Uz'], np.float32),
        np.asarray(inputs['Ur'], np.float32), np.asarray(inputs['Uh'], np.float32),
        np.asarray(inputs['bz'], np.float32), np.asarray(inputs['br'], np.float32),
        np.asarray(inputs['bh'], np.float32), table0)

    nc = _build_nc(hd)
    res = _run(hd, nc)
    dh_cores = [np.asarray(res.results[c]["dh"], np.float32) for c in range(8)]
    return _assemble(hd, dh_cores, table0)


if __name__ == "__main__":
    d = np.load('/tmp/ref_inputs.npz')
    inputs = {k: d[k] for k in d.files}
    got = kernel(**inputs)
    exp = np.load('/tmp/ref_out_np.npy')
    err = np.abs(got - exp).max()
    print("abs err:", err, "rel:", err / np.abs(exp).max())


# revision 13
# speedup vs baseline: 1.6602x; 1.1980x over previous
"""Trainium2 Bass kernel for nn_AtBatCell: GRU recurrence over a shared state
table with gather/scatter-add per timestep.

Strategy: steps touching disjoint table rows are independent, so the T=8192
sequential scan collapses into waves (levels of the row-dependency DAG).
The device runs the first DW waves (87.5% of steps) as fully-packed batches
of 128-step GRU chunks; the small high-level tail (steps whose row chains
are 3+ deep) is finished on the host together with the delta assembly the
host already performs.

Device schedule (per core, SPMD-identical):
 - wave 1: all rows are first touches -> host-packed contiguous stream,
   plain DMA, no per-row descriptors. Steps whose rows are re-read by
   wave 2 are sorted first and their deltas scatter-added (SWDGE) into a
   small gather table G (~512 rows).
 - wave 2: rows come via dma_gather from G.
 - matmuls run in bf16 on the PE (f32 PSUM accumulate); H and r*h are
   PE-transposed in bf16.
 - deltas (dh) ship to DRAM contiguously; the host applies them and then
   computes the remaining tail waves directly (row chains are disjoint
   within a wave, so the tail is a few batched GEMMs).

Chunks are filled to exactly 128 steps by delaying "free" steps (steps no
later device step depends on) to later waves; component-based core
assignment keeps all touches of a row on one core."""
import os
import sys
for _p in ('/opt/trn_rl_repo', '/root/.axon_site/_ro/trn_rl_repo'):
    if os.path.isdir(_p) and _p not in sys.path:
        sys.path.insert(0, _p)

import collections
import numpy as np

SIT = 64
S = 256
S2 = 512
CHUNK = 128          # steps per compute chunk
SPARE = 128          # spare zero rows absorbing padding/dup scatters
BLOCK = 4            # chunks per gather/scatter/dh block
NCORES = 8
DW = 2               # device waves; later waves are finished on host
K_CAP = [5, 2]       # chunks per wave per core


def _schedule(b, p, n_rows_total):
    T = len(b)
    bl = b.astype(np.int64)
    pl = p.astype(np.int64)
    last = np.zeros(n_rows_total, np.int64)
    lev = np.empty(T, np.int64)
    for t in range(T):
        lv = max(last[bl[t]], last[pl[t]]) + 1
        lev[t] = lv
        last[bl[t]] = lv
        last[pl[t]] = lv

    rowtouch = collections.defaultdict(list)
    for t in range(T):
        rowtouch[bl[t]].append((t, 0))
        rowtouch[pl[t]].append((t, 1))
    nxt = np.full((T, 2), -1, np.int64)
    first = np.zeros((T, 2), bool)
    for r, lst in rowtouch.items():
        first[lst[0][0], lst[0][1]] = True
        for (t1, s1), (t2, _) in zip(lst, lst[1:]):
            nxt[t1, s1] = t2

    prov_dev = lev <= DW
    free = np.zeros(T, bool)
    for t in range(T):
        if not prov_dev[t]:
            continue
        free[t] = all(
            nxt[t, s] < 0 or not prov_dev[nxt[t, s]] for s in (0, 1))

    # union-find over provisional device steps
    parent = np.arange(T)

    def find(a):
        while parent[a] != a:
            parent[a] = parent[parent[a]]
            a = parent[a]
        return a

    for t in range(T):
        if not prov_dev[t]:
            continue
        for s in (0, 1):
            t2 = nxt[t, s]
            if t2 >= 0 and prov_dev[t2]:
                ra, rb = find(t), find(t2)
                if ra != rb:
                    parent[ra] = rb
    comp = collections.defaultdict(list)
    for t in range(T):
        if prov_dev[t]:
            comp[find(t)].append(t)
    comps = sorted(comp.values(), key=lambda v: (-len(v), v[0]))

    # balance components across cores on (per-level counts, total)
    targets = np.zeros(DW + 1)
    cvecs = []
    for cv in comps:
        v = np.zeros(DW + 1)
        for t in cv:
            v[lev[t] - 1] += 1
        v[DW] = len(cv)
        cvecs.append(v)
        targets += v
    targets = np.maximum(targets / NCORES, 1e-9)
    loads = np.zeros((NCORES, DW + 1))
    cassign = {}
    for cv, v in zip(comps, cvecs):
        cidx = int(np.argmin(((loads + v) / targets).max(axis=1)))
        loads[cidx] += v
        cassign[cv[0]] = cidx

    # per-core wave placement: nonfree at their level, free fill remaining
    # capacity (any wave >= their level), overflow goes to the host tail
    wave_steps = [[[] for _ in range(DW)] for _ in range(NCORES)]
    for cv in comps:
        c = cassign[cv[0]]
        for t in cv:
            if not free[t]:
                wave_steps[c][lev[t] - 1].append(t)
    for c in range(NCORES):
        for w in range(DW):
            assert len(wave_steps[c][w]) <= K_CAP[w] * CHUNK, \
                f"core {c} wave {w}: nonfree overflow"
    for cv in comps:
        c = cassign[cv[0]]
        for t in cv:
            if not free[t]:
                continue
            for w in range(int(lev[t]) - 1, DW):
                if len(wave_steps[c][w]) < K_CAP[w] * CHUNK:
                    wave_steps[c][w].append(t)
                    break
            # else: host tail

    dev_mask = np.zeros(T, bool)
    for c in range(NCORES):
        for w in range(DW):
            for t in wave_steps[c][w]:
                dev_mask[t] = True

    keep = np.zeros((T, 2), bool)
    for t in range(T):
        if dev_mask[t]:
            for s in (0, 1):
                keep[t, s] = nxt[t, s] >= 0 and dev_mask[nxt[t, s]]

    # keep-steps first within each wave (scatter prefix)
    for c in range(NCORES):
        for w in range(DW):
            wave_steps[c][w].sort(key=lambda t: (not keep[t].any(), t))

    host_steps = np.nonzero(~dev_mask)[0]

    # invariants
    for r, lst in rowtouch.items():
        seen_host = False
        for (t, s) in lst:
            if dev_mask[t]:
                assert not seen_host
            else:
                seen_host = True
    for c in range(NCORES):
        for t in wave_steps[c][0]:
            assert first[t].all(), "non-fresh slot in wave 1"

    return dict(lev=lev, nxt=nxt, first=first, keep=keep,
                wave_steps=wave_steps, host_steps=host_steps,
                dev_mask=dev_mask)


def _build_host_data(x, b, p, Wz, Wr, Wh, Uz, Ur, Uh, bz, br, bh, table0):
    import ml_dtypes
    bf16 = ml_dtypes.bfloat16
    N = table0.shape[0]
    b = b.astype(np.int64)
    p = p.astype(np.int64)
    sch = _schedule(b, p, N)
    keep, first = sch['keep'], sch['first']
    wave_steps = sch['wave_steps']

    wave_chunks = list(K_CAP)
    n_chunks = sum(wave_chunks)
    T_pad = n_chunks * CHUNK

    # scatter prefix: chunks holding keep-steps in wave 1..DW-1
    kc_wave = [0] * DW
    for w in range(DW - 1):
        mx = max(sum(1 for t in wave_steps[c][w] if keep[t].any())
                 for c in range(NCORES))
        kc_wave[w] = -(-mx // CHUNK)

    # blocks: (chunk_start, n_chunks, wave, all_fresh, keep_chunks, fresh_off)
    blocks = []
    fc = 0
    cl = 0
    for w, wc in enumerate(wave_chunks):
        for bstart in range(0, wc, BLOCK):
            nb = min(BLOCK, wc - bstart)
            af = (w == 0)
            kc = max(0, min(nb, kc_wave[w] - bstart))
            fo = -1
            if af:
                fo = fc
                fc += 2 * nb
            blocks.append((cl + bstart, nb, w, af, kc, fo))
        cl += wc
    fresh_cols = max(fc, 2)
    chunk_wave = np.repeat(np.arange(DW), wave_chunks)

    # per-core data
    per_core = []
    dup_any = False
    for c in range(NCORES):
        ob = np.full(T_pad, -1, np.int64)   # original row ids (host assembly)
        op = np.full(T_pad, -1, np.int64)
        x_c = np.zeros((T_pad, SIT), np.float32)
        bias_c = np.zeros(T_pad, np.float32)
        dm_c = np.zeros(T_pad, np.float32)
        st_c = np.full(T_pad, -1, np.int64)
        j0 = 0
        for w, wc in enumerate(wave_chunks):
            ts = wave_steps[c][w]
            sl = slice(j0, j0 + len(ts))
            tsa = np.asarray(ts, np.int64)
            if len(ts):
                st_c[sl] = tsa
                ob[sl] = b[tsa]
                op[sl] = p[tsa]
                x_c[sl] = x[tsa]
                bias_c[sl] = 1.0
            j0 += wc * CHUNK
        dup = (ob == op) & (ob >= 0)
        if dup.any():
            dup_any = True
            dm_c[dup] = 1.0
            op[dup] = -1          # p-side folded into b via dupmask
            assert False, "dup steps unsupported with SBUF-dst scatter"
        per_core.append(dict(ob=ob, op=op, x_c=x_c, bias_c=bias_c,
                             dm_c=dm_c, st=st_c))

    g0 = K_CAP[0] * CHUNK
    for c in range(NCORES):
        pc = per_core[c]
        ob, op, st = pc['ob'], pc['op'], pc['st']
        # consumer-slot scatter idx: producer row's dh lands directly in the
        # wave-2 H tile slot of its next (device) touch. Parity-split SBUF
        # scatter: even slots -> consumer tile, odd slots -> trash tile.
        slot_of = {int(t): q for q, t in enumerate(st) if t >= 0}
        sides = np.stack([ob, op], axis=1)
        bs = np.zeros(T_pad, np.int64)
        ps = np.zeros(T_pad, np.int64)
        for q in range(min(kc_wave[0] * CHUNK, T_pad)):
            t = int(st[q])
            j = q % CHUNK
            for side, arr in ((0, bs), (1, ps)):
                idxv = 256 * (j % 4) + 128 + j        # odd parity -> trash
                if t >= 0 and keep[t, side]:
                    t2 = int(sch['nxt'][t, side])
                    q2 = slot_of[t2]
                    assert q2 >= g0, "consumer not in wave 2"
                    row = sides[q, side]
                    if pc['ob'][q2] == row:
                        side2 = 0
                    else:
                        assert pc['op'][q2] == row
                        side2 = 1
                    col2 = 2 * (q2 // CHUNK - wave_chunks[0]) + side2
                    idxv = col2 * 256 + (q2 % CHUNK)  # even parity -> consumer
                arr[q] = idxv
        idx_il = np.stack([bs.reshape(-1, CHUNK), ps.reshape(-1, CHUNK)],
                          axis=1).reshape(-1).astype(np.int16)
        idx_rep = np.tile(idx_il.reshape(-1, 16).T, (8, 1)).copy()
        xT_c = np.zeros((SIT + 1, T_pad), np.float32)
        xT_c[:SIT] = pc['x_c'].T
        xT_c[SIT] = pc['bias_c']
        # fresh stream: wave-1 chunks, zeros for pads; shipped bf16 both in
        # natural [step, 2S] layout and pre-transposed [state, step] layout
        # (the transposed copy feeds the PE stationary directly)
        fresh_c = np.zeros((128, fresh_cols, S), np.float32)
        for (cs, nb, w, af, kc, fo) in blocks:
            if fo < 0:
                continue
            for q in range(nb):
                sl = slice((cs + q) * CHUNK, (cs + q + 1) * CHUNK)
                vb = pc['ob'][sl] >= 0
                vp = pc['op'][sl] >= 0
                fresh_c[vb, fo + 2 * q, :] = table0[pc['ob'][sl][vb]]
                fresh_c[vp, fo + 2 * q + 1, :] = table0[pc['op'][sl][vp]]
        fresh_b16 = fresh_c.astype(bf16)
        k1 = wave_chunks[0]
        freshT = np.zeros((128, k1, 4, CHUNK), bf16)
        for q in range(k1):
            hcat = np.concatenate([fresh_b16[:, 2 * q, :],
                                   fresh_b16[:, 2 * q + 1, :]], axis=1)
            for k in range(4):
                freshT[:, q, k, :] = hcat[:, CHUNK * k:CHUNK * (k + 1)].T
        # wave-2 H tile pre-fill: table0 value of every referenced row
        # (scattered dh accumulates on top to form the post-wave-1 value)
        k2 = wave_chunks[1]
        fresh2 = np.zeros((128, 2 * k2, S), np.float32)
        for q in range(g0, T_pad):
            if st[q] < 0:
                continue
            cc = q // CHUNK - wave_chunks[0]
            j = q % CHUNK
            fresh2[j, 2 * cc, :] = table0[ob[q]]
            fresh2[j, 2 * cc + 1, :] = table0[op[q]]
        dmask_c = pc['dm_c'].reshape(n_chunks, CHUNK).T.copy()
        per_core[c] = dict(idx_rep=idx_rep, xT=xT_c.astype(bf16),
                           fresh_arr=fresh_b16, freshT=freshT,
                           fresh2=fresh2.astype(bf16), dmask=dmask_c,
                           ob=ob, op=op)

    WzT = np.concatenate([Wz.T, bz[None, :]], axis=0)
    WrT = np.concatenate([Wr.T, -br[None, :]], axis=0)
    WhT = np.concatenate([Wh.T, bh[None, :]], axis=0)

    def ut(U):
        return np.ascontiguousarray(U.T.reshape(4, 128, S2).transpose(1, 0, 2))

    hd = dict(
        WzT=WzT.astype(bf16), WrT=WrT.astype(bf16), WhT=WhT.astype(bf16),
        UzT=ut(Uz).astype(bf16), UrT=ut(Ur).astype(bf16),
        UhT=ut(Uh).astype(bf16),
        n_chunks=n_chunks, blocks=blocks, fresh_cols=fresh_cols,
        kc_wave=kc_wave, wave_chunks=wave_chunks, chunk_wave=chunk_wave,
        T_pad=T_pad, dup_any=dup_any,
        per_core=per_core,
        host_steps=sch['host_steps'], lev=sch['lev'],
        x=x, b=b, p=p, Wz=Wz, Wr=Wr, Wh=Wh, Uz=Uz, Ur=Ur, Uh=Uh,
        bz=bz, br=br, bh=bh,
    )
    return hd


def _build_nc(hd):
    import concourse.bacc as bacc
    import concourse.mybir as mybir
    import concourse.tile as tile
    from concourse.masks import make_identity

    n_chunks = hd['n_chunks']
    T_pad = hd['T_pad']
    blocks = hd['blocks']
    f32 = mybir.dt.float32
    bf16 = mybir.dt.bfloat16
    i16 = mybir.dt.int16

    nc = bacc.Bacc("TRN2", target_bir_lowering=False, debug=True)

    idx_in = nc.dram_tensor("idx", (128, 2 * T_pad // 16), i16, kind="ExternalInput")
    fresh_in = nc.dram_tensor("fresh", (128, hd['fresh_cols'], S), bf16,
                              kind="ExternalInput")
    fresh2_in = nc.dram_tensor("fresh2", (128, 2 * hd['wave_chunks'][1], S),
                               bf16, kind="ExternalInput")
    freshT_in = nc.dram_tensor("freshT", (128, hd['wave_chunks'][0], 4, CHUNK),
                               bf16, kind="ExternalInput")
    xT_in = nc.dram_tensor("xT", (SIT + 1, T_pad), bf16, kind="ExternalInput")
    WzT_in = nc.dram_tensor("WzT", (SIT + 1, S2), bf16, kind="ExternalInput")
    WrT_in = nc.dram_tensor("WrT", (SIT + 1, S2), bf16, kind="ExternalInput")
    WhT_in = nc.dram_tensor("WhT", (SIT + 1, S2), bf16, kind="ExternalInput")
    UzT_in = nc.dram_tensor("UzT", (128, 4, S2), bf16, kind="ExternalInput")
    UrT_in = nc.dram_tensor("UrT", (128, 4, S2), bf16, kind="ExternalInput")
    UhT_in = nc.dram_tensor("UhT", (128, 4, S2), bf16, kind="ExternalInput")
    dmask_in = nc.dram_tensor("dmask", (128, n_chunks), f32, kind="ExternalInput")

    dh_out = nc.dram_tensor("dh", (128, 2 * n_chunks, S), bf16,
                            kind="ExternalOutput")

    Sig = mybir.ActivationFunctionType.Sigmoid
    Tanh = mybir.ActivationFunctionType.Tanh

    wave_chunks = hd['wave_chunks']
    kc_wave = hd['kc_wave']
    chunk_wave = hd['chunk_wave']

    with tile.TileContext(nc) as tc:
        with tc.tile_pool(name="const", bufs=1) as cpool, \
             tc.tile_pool(name="gath", bufs=8) as gpool, \
             tc.tile_pool(name="dhb", bufs=4) as dhpool, \
             tc.tile_pool(name="work", bufs=4) as wpool, \
             tc.tile_pool(name="psA", bufs=2, space="PSUM") as psA, \
             tc.tile_pool(name="psZ", bufs=2, space="PSUM") as psZ, \
             tc.tile_pool(name="psR", bufs=2, space="PSUM") as psR, \
             tc.tile_pool(name="psM", bufs=2, space="PSUM") as psM:

            # ---- static loads (sync HWDGE), most-urgent first ----
            xT_sb = cpool.tile([SIT + 1, T_pad], bf16, tag="xT")
            nc.sync.dma_start(xT_sb[:], xT_in[:])
            w_sb = {}
            for nm, t in (("WzT", WzT_in), ("WrT", WrT_in)):
                w_sb[nm] = cpool.tile([SIT + 1, S2], bf16, tag=nm, name=nm + "_sb")
                nc.sync.dma_start(w_sb[nm][:], t[:])
            for nm, t in (("UzT", UzT_in), ("UrT", UrT_in)):
                w_sb[nm] = cpool.tile([128, 4, S2], bf16, tag=nm, name=nm + "_sb")
                nc.sync.dma_start(w_sb[nm][:], t[:])
            gtiles = {}
            httiles = {}

            def emit_fresh(c):
                g = gpool.tile([128, 2, S], bf16, tag="hg", name=f"hg_{c}")
                nc.sync.dma_start(g[:], fresh_in[:, 2 * c:2 * c + 2, :])
                ht = gpool.tile([128, 4, CHUNK], bf16, tag="hT", name=f"hT_{c}")
                nc.sync.dma_start(ht[:], freshT_in[:, c, :, :])
                return g, ht

            # first chunk's rows before the remaining weights
            gtiles[0], httiles[0] = emit_fresh(0)
            w_sb["WhT"] = cpool.tile([SIT + 1, S2], bf16, tag="WhT", name="WhT_sb")
            nc.sync.dma_start(w_sb["WhT"][:], WhT_in[:])
            w_sb["UhT"] = cpool.tile([128, 4, S2], bf16, tag="UhT", name="UhT_sb")
            nc.sync.dma_start(w_sb["UhT"][:], UhT_in[:])
            # wave-2 H tile: host pre-fill (table0 rows); wave-1 scatters
            # accumulate dh directly into it (parity-split SBUF scatter)
            k2 = wave_chunks[1]
            cons = cpool.tile([128, 2 * k2, S], bf16, tag="cons")
            nc.sync.dma_start(cons[:], fresh2_in[:])
            trash = cpool.tile([128, 2 * k2, S], bf16, tag="trash")
            nc.scalar.memzero(trash[:])
            for c in range(1, wave_chunks[0]):
                gtiles[c], httiles[c] = emit_fresh(c)
            idx_sb = cpool.tile([128, 2 * T_pad // 16], i16, tag="idx")
            nc.sync.dma_start(idx_sb[:], idx_in[:])
            dmask_sb = cpool.tile([128, n_chunks], f32, tag="dmask")
            if hd['dup_any']:
                nc.sync.dma_start(dmask_sb[:], dmask_in[:])
            identb = cpool.tile([128, 128], bf16, tag="identb")
            make_identity(nc, identb[:])

            for c in range(n_chunks):
                w = int(chunk_wave[c])
                if w == 0:
                    g = gtiles.pop(c)[:]
                else:
                    cw = c - wave_chunks[0]
                    g = cons[:, 2 * cw:2 * cw + 2, :]
                hg2 = g.rearrange("p a b -> p (a b)")
                dhb = dhpool.tile([128, 2, S], bf16, tag="dh", name=f"dh_{c}")

                tr_ps_f = psA.tile([128, 4, CHUNK], f32, tag="tr",
                                   name=f"trp_{c}")
                tr_ps = tr_ps_f[:].bitcast(bf16)
                if c in httiles:
                    ht = httiles.pop(c)      # host-packed transposed stream
                else:
                    # PE transpose of the gathered (bf16) rows
                    ht_ps = tr_ps[:, :, 0:CHUNK]
                    for k in range(4):
                        nc.tensor.transpose(
                            ht_ps[:, k, :], hg2[:, CHUNK * k:CHUNK * (k + 1)],
                            identb[:])
                    ht = wpool.tile([128, 4, CHUNK], bf16, tag="ht")
                    nc.vector.tensor_copy(ht[:], ht_ps)

                xt_c = xT_sb[:, CHUNK * c:CHUNK * (c + 1)]

                zpre = psZ.tile([128, S2], f32, tag="zpre")
                rpre = psR.tile([128, S2], f32, tag="rpre")
                nc.tensor.matmul(zpre[:], xt_c, w_sb["WzT"][:],
                                 start=True, stop=False)
                nc.tensor.matmul(rpre[:], xt_c, w_sb["WrT"][:],
                                 start=True, stop=False)
                for k in range(4):
                    nc.tensor.matmul(zpre[:], ht[:, k, :], w_sb["UzT"][:, k, :],
                                     start=False, stop=(k == 3))
                    nc.tensor.matmul(rpre[:], ht[:, k, :], w_sb["UrT"][:, k, :],
                                     start=False, stop=(k == 3))

                zc = wpool.tile([128, S2], f32, tag="zc")
                r = wpool.tile([128, S2], f32, tag="r")
                nc.scalar.activation(zc[:], zpre[:], Sig, scale=-1.0)  # 1-z
                nc.scalar.activation(r[:], rpre[:], Sig)

                rh = wpool.tile([128, S2], bf16, tag="rh")
                nc.vector.tensor_mul(rh[:], r[:], hg2)
                rht_ps = tr_ps[:, :, CHUNK:2 * CHUNK]
                for k in range(4):
                    nc.tensor.transpose(
                        rht_ps[:, k, :], rh[:, CHUNK * k:CHUNK * (k + 1)],
                        identb[:])
                rht = wpool.tile([128, 4, CHUNK], bf16, tag="rht")
                nc.vector.tensor_copy(rht[:], rht_ps)

                mpre = psM.tile([128, S2], f32, tag="mpre")
                nc.tensor.matmul(mpre[:], xt_c, w_sb["WhT"][:],
                                 start=True, stop=False)
                for k in range(4):
                    nc.tensor.matmul(mpre[:], rht[:, k, :], w_sb["UhT"][:, k, :],
                                     start=False, stop=(k == 3))

                m = wpool.tile([128, S2], f32, tag="m")
                nc.scalar.activation(m[:], mpre[:], Tanh)

                # dh = (1-z)*(m-h)
                t1 = wpool.tile([128, S2], f32, tag="t1")
                nc.vector.tensor_sub(t1[:], m[:], hg2)
                dh_view = dhb[:].rearrange("p a b -> p (a b)")
                nc.vector.tensor_mul(dh_view, zc[:], t1[:])
                if hd['dup_any']:
                    tm = wpool.tile([128, S], bf16, tag="tm")
                    nc.vector.tensor_scalar_mul(
                        tm[:], dhb[:, 1, :], dmask_sb[:, c:c + 1])
                    nc.vector.tensor_add(
                        dhb[:, 0, :], dhb[:, 0, :], tm[:])

                # ship deltas to host (sync HWDGE)
                nc.sync.dma_start(dh_out[:, 2 * c:2 * c + 2, :], dhb[:])
                cw0 = c - int(np.sum(wave_chunks[:w]))
                if w < DW - 1 and cw0 < kc_wave[w]:
                    nc.gpsimd.dma_scatter_add(
                        cons[:], dhb[:],
                        idx_sb[:, 16 * c:16 * (c + 1)],
                        2 * CHUNK, 2 * CHUNK, S, queue_num=0,
                        sbuf_tokens_per_rank=128, parity_reg=0,
                        out_ap_other=trash[:],
                    )

    nc.compile()
    return nc


def _in_map(hd, core):
    pc = hd['per_core'][core]
    return {
        "idx": pc['idx_rep'], "fresh": pc['fresh_arr'],
        "freshT": pc['freshT'], "fresh2": pc['fresh2'], "xT": pc['xT'],
        "WzT": hd['WzT'], "WrT": hd['WrT'], "WhT": hd['WhT'],
        "UzT": hd['UzT'], "UrT": hd['UrT'], "UhT": hd['UhT'],
        "dmask": pc['dmask'],
    }


def _run(hd, nc, trace=False):
    from concourse.bass_utils import run_bass_kernel_spmd
    return run_bass_kernel_spmd(nc, [_in_map(hd, c) for c in range(8)],
                                list(range(8)), trace=trace)


def _assemble(hd, dh_cores, table0):
    """Apply device deltas (rows never cross cores), then finish the tail
    waves on host (same-level steps never share a row -> batched GEMMs)."""
    n_chunks = hd['n_chunks']
    out = table0.astype(np.float32).copy()
    for cidx in range(8):
        dh = np.ascontiguousarray(dh_cores[cidx].transpose(1, 0, 2))
        dh = dh.reshape(n_chunks, 2, CHUNK, S).transpose(0, 2, 1, 3)
        dh = dh.reshape(hd['T_pad'] * 2, S)
        pc = hd['per_core'][cidx]
        rows = np.stack([pc['ob'], pc['op']], axis=1).reshape(-1)
        valid = rows >= 0
        np.add.at(out, rows[valid], dh[valid])

    hs = np.asarray(hd['host_steps'], np.int64)
    if len(hs):
        x, b, p = hd['x'], hd['b'], hd['p']
        Wz, Wr, Wh = hd['Wz'], hd['Wr'], hd['Wh']
        Uz, Ur, Uh = hd['Uz'], hd['Ur'], hd['Uh']
        bz, br, bh = hd['bz'], hd['br'], hd['bh']
        levs = hd['lev'][hs]
        for L in np.unique(levs):
            ts = hs[levs == L]
            H = np.concatenate([out[b[ts]], out[p[ts]]], axis=1)
            Z = 1 / (1 + np.exp(-(x[ts] @ Wz.T + H @ Uz.T + bz)))
            R = 1 / (1 + np.exp(-(x[ts] @ Wr.T + H @ Ur.T - br)))
            M = np.tanh(x[ts] @ Wh.T + (R * H) @ Uh.T + bh)
            dh = (1.0 - Z) * (M - H)
            np.add.at(out, b[ts], dh[:, :S])
            np.add.at(out, p[ts], dh[:, S:])
    return out


def kernel(**inputs):
    x = np.asarray(inputs['x'], dtype=np.float32)
    b = np.asarray(inputs['b'])
    p = np.asarray(inputs['p'])
    table0 = np.asarray(inputs['table0'], dtype=np.float32)

    hd = _build_host_data(
        x, b, p,
        np.asarray(inputs['Wz'], np.float32), np.asarray(inputs['Wr'], np.float32),
        np.asarray(inputs['Wh'], np.float32), np.asarray(inputs['Uz'], np.float32),
        np.asarray(inputs['Ur'], np.float32), np.asarray(inputs['Uh'], np.float32),
        np.asarray(inputs['bz'], np.float32), np.asarray(inputs['br'], np.float32),
        np.asarray(inputs['bh'], np.float32), table0)

    nc = _build_nc(hd)
    res = _run(hd, nc)
    dh_cores = [np.asarray(res.results[c]["dh"], np.float32) for c in range(8)]
    return _assemble(hd, dh_cores, table0)


if __name__ == "__main__":
    d = np.load('/tmp/ref_inputs.npz')
    inputs = {k: d[k] for k in d.files}
    got = kernel(**inputs)
    exp = np.load('/tmp/ref_out_np.npy')
    err = np.abs(got - exp).max()
    print("abs err:", err, "rel:", err / np.abs(exp).max())


# revision 14
# speedup vs baseline: 1.9178x; 1.1551x over previous
"""Trainium2 Bass kernel for nn_AtBatCell: GRU recurrence over a shared state
table with gather/scatter-add per timestep.

Strategy: steps touching disjoint table rows are independent, so the T=8192
sequential scan collapses into waves (levels of the row-dependency DAG).
The device runs the first DW waves (87.5% of steps) as fully-packed batches
of 128-step GRU chunks; the small high-level tail (steps whose row chains
are 3+ deep) is finished on the host together with the delta assembly the
host already performs.

Device schedule (per core, SPMD-identical):
 - wave 1: all rows are first touches -> host-packed contiguous stream,
   plain DMA, no per-row descriptors. Steps whose rows are re-read by
   wave 2 are sorted first and their deltas scatter-added (SWDGE) into a
   small gather table G (~512 rows).
 - wave 2: rows come via dma_gather from G.
 - matmuls run in bf16 on the PE (f32 PSUM accumulate); H and r*h are
   PE-transposed in bf16.
 - deltas (dh) ship to DRAM contiguously; the host applies them and then
   computes the remaining tail waves directly (row chains are disjoint
   within a wave, so the tail is a few batched GEMMs).

Chunks are filled to exactly 128 steps by delaying "free" steps (steps no
later device step depends on) to later waves; component-based core
assignment keeps all touches of a row on one core."""
import os
import sys
for _p in ('/opt/trn_rl_repo', '/root/.axon_site/_ro/trn_rl_repo'):
    if os.path.isdir(_p) and _p not in sys.path:
        sys.path.insert(0, _p)

import collections
import numpy as np

SIT = 64
S = 256
S2 = 512
CHUNK = 128          # steps per compute chunk
SPARE = 128          # spare zero rows absorbing padding/dup scatters
BLOCK = 4            # chunks per gather/scatter/dh block
NCORES = 8
DW = 2               # device waves; later waves are finished on host
K_CAP = [5, 2]       # chunks per wave per core


def _schedule(b, p, n_rows_total):
    T = len(b)
    bl = b.astype(np.int64)
    pl = p.astype(np.int64)
    last = np.zeros(n_rows_total, np.int64)
    lev = np.empty(T, np.int64)
    for t in range(T):
        lv = max(last[bl[t]], last[pl[t]]) + 1
        lev[t] = lv
        last[bl[t]] = lv
        last[pl[t]] = lv

    rowtouch = collections.defaultdict(list)
    for t in range(T):
        rowtouch[bl[t]].append((t, 0))
        rowtouch[pl[t]].append((t, 1))
    nxt = np.full((T, 2), -1, np.int64)
    first = np.zeros((T, 2), bool)
    for r, lst in rowtouch.items():
        first[lst[0][0], lst[0][1]] = True
        for (t1, s1), (t2, _) in zip(lst, lst[1:]):
            nxt[t1, s1] = t2

    prov_dev = lev <= DW
    free = np.zeros(T, bool)
    for t in range(T):
        if not prov_dev[t]:
            continue
        free[t] = all(
            nxt[t, s] < 0 or not prov_dev[nxt[t, s]] for s in (0, 1))

    # union-find over provisional device steps
    parent = np.arange(T)

    def find(a):
        while parent[a] != a:
            parent[a] = parent[parent[a]]
            a = parent[a]
        return a

    for t in range(T):
        if not prov_dev[t]:
            continue
        for s in (0, 1):
            t2 = nxt[t, s]
            if t2 >= 0 and prov_dev[t2]:
                ra, rb = find(t), find(t2)
                if ra != rb:
                    parent[ra] = rb
    comp = collections.defaultdict(list)
    for t in range(T):
        if prov_dev[t]:
            comp[find(t)].append(t)
    comps = sorted(comp.values(), key=lambda v: (-len(v), v[0]))

    # balance components across cores on (per-level counts, total)
    targets = np.zeros(DW + 1)
    cvecs = []
    for cv in comps:
        v = np.zeros(DW + 1)
        for t in cv:
            v[lev[t] - 1] += 1
        v[DW] = len(cv)
        cvecs.append(v)
        targets += v
    targets = np.maximum(targets / NCORES, 1e-9)
    loads = np.zeros((NCORES, DW + 1))
    cassign = {}
    for cv, v in zip(comps, cvecs):
        cidx = int(np.argmin(((loads + v) / targets).max(axis=1)))
        loads[cidx] += v
        cassign[cv[0]] = cidx

    # per-core wave placement: nonfree at their level, free fill remaining
    # capacity (any wave >= their level), overflow goes to the host tail
    wave_steps = [[[] for _ in range(DW)] for _ in range(NCORES)]
    for cv in comps:
        c = cassign[cv[0]]
        for t in cv:
            if not free[t]:
                wave_steps[c][lev[t] - 1].append(t)
    for c in range(NCORES):
        for w in range(DW):
            assert len(wave_steps[c][w]) <= K_CAP[w] * CHUNK, \
                f"core {c} wave {w}: nonfree overflow"
    for cv in comps:
        c = cassign[cv[0]]
        for t in cv:
            if not free[t]:
                continue
            for w in range(int(lev[t]) - 1, DW):
                if len(wave_steps[c][w]) < K_CAP[w] * CHUNK:
                    wave_steps[c][w].append(t)
                    break
            # else: host tail

    dev_mask = np.zeros(T, bool)
    for c in range(NCORES):
        for w in range(DW):
            for t in wave_steps[c][w]:
                dev_mask[t] = True

    keep = np.zeros((T, 2), bool)
    for t in range(T):
        if dev_mask[t]:
            for s in (0, 1):
                keep[t, s] = nxt[t, s] >= 0 and dev_mask[nxt[t, s]]

    # keep-steps first within each wave (scatter prefix)
    for c in range(NCORES):
        for w in range(DW):
            wave_steps[c][w].sort(key=lambda t: (not keep[t].any(), t))

    host_steps = np.nonzero(~dev_mask)[0]

    # invariants
    for r, lst in rowtouch.items():
        seen_host = False
        for (t, s) in lst:
            if dev_mask[t]:
                assert not seen_host
            else:
                seen_host = True
    for c in range(NCORES):
        for t in wave_steps[c][0]:
            assert first[t].all(), "non-fresh slot in wave 1"

    return dict(lev=lev, nxt=nxt, first=first, keep=keep,
                wave_steps=wave_steps, host_steps=host_steps,
                dev_mask=dev_mask)


def _build_host_data(x, b, p, Wz, Wr, Wh, Uz, Ur, Uh, bz, br, bh, table0):
    import ml_dtypes
    bf16 = ml_dtypes.bfloat16
    N = table0.shape[0]
    b = b.astype(np.int64)
    p = p.astype(np.int64)
    sch = _schedule(b, p, N)
    keep, first = sch['keep'], sch['first']
    wave_steps = sch['wave_steps']

    wave_chunks = list(K_CAP)
    n_chunks = sum(wave_chunks)
    T_pad = n_chunks * CHUNK

    # scatter prefix: chunks holding keep-steps in wave 1..DW-1
    kc_wave = [0] * DW
    for w in range(DW - 1):
        mx = max(sum(1 for t in wave_steps[c][w] if keep[t].any())
                 for c in range(NCORES))
        kc_wave[w] = -(-mx // CHUNK)

    # blocks: (chunk_start, n_chunks, wave, all_fresh, keep_chunks, fresh_off)
    blocks = []
    fc = 0
    cl = 0
    for w, wc in enumerate(wave_chunks):
        for bstart in range(0, wc, BLOCK):
            nb = min(BLOCK, wc - bstart)
            af = (w == 0)
            kc = max(0, min(nb, kc_wave[w] - bstart))
            fo = -1
            if af:
                fo = fc
                fc += 2 * nb
            blocks.append((cl + bstart, nb, w, af, kc, fo))
        cl += wc
    fresh_cols = max(fc, 2)
    chunk_wave = np.repeat(np.arange(DW), wave_chunks)

    # per-core data
    per_core = []
    dup_any = False
    for c in range(NCORES):
        ob = np.full(T_pad, -1, np.int64)   # original row ids (host assembly)
        op = np.full(T_pad, -1, np.int64)
        x_c = np.zeros((T_pad, SIT), np.float32)
        bias_c = np.zeros(T_pad, np.float32)
        dm_c = np.zeros(T_pad, np.float32)
        st_c = np.full(T_pad, -1, np.int64)
        j0 = 0
        for w, wc in enumerate(wave_chunks):
            ts = wave_steps[c][w]
            sl = slice(j0, j0 + len(ts))
            tsa = np.asarray(ts, np.int64)
            if len(ts):
                st_c[sl] = tsa
                ob[sl] = b[tsa]
                op[sl] = p[tsa]
                x_c[sl] = x[tsa]
                bias_c[sl] = 1.0
            j0 += wc * CHUNK
        dup = (ob == op) & (ob >= 0)
        if dup.any():
            dup_any = True
            dm_c[dup] = 1.0
            op[dup] = -1          # p-side folded into b via dupmask
            assert False, "dup steps unsupported with SBUF-dst scatter"
        per_core.append(dict(ob=ob, op=op, x_c=x_c, bias_c=bias_c,
                             dm_c=dm_c, st=st_c))

    g0 = K_CAP[0] * CHUNK
    for c in range(NCORES):
        pc = per_core[c]
        ob, op, st = pc['ob'], pc['op'], pc['st']
        # consumer-slot scatter idx: producer row's dh lands directly in the
        # wave-2 H tile slot of its next (device) touch. Parity-split SBUF
        # scatter: even slots -> consumer tile, odd slots -> trash tile.
        slot_of = {int(t): q for q, t in enumerate(st) if t >= 0}
        sides = np.stack([ob, op], axis=1)
        bs = np.zeros(T_pad, np.int64)
        ps = np.zeros(T_pad, np.int64)
        for q in range(min(kc_wave[0] * CHUNK, T_pad)):
            t = int(st[q])
            j = q % CHUNK
            for side, arr in ((0, bs), (1, ps)):
                idxv = 256 * (j % 4) + 128 + j        # odd parity -> trash
                if t >= 0 and keep[t, side]:
                    t2 = int(sch['nxt'][t, side])
                    q2 = slot_of[t2]
                    assert q2 >= g0, "consumer not in wave 2"
                    row = sides[q, side]
                    if pc['ob'][q2] == row:
                        side2 = 0
                    else:
                        assert pc['op'][q2] == row
                        side2 = 1
                    col2 = 2 * (q2 // CHUNK - wave_chunks[0]) + side2
                    idxv = col2 * 256 + (q2 % CHUNK)  # even parity -> consumer
                arr[q] = idxv
        idx_il = np.stack([bs.reshape(-1, CHUNK), ps.reshape(-1, CHUNK)],
                          axis=1).reshape(-1).astype(np.int16)
        idx_rep = np.tile(idx_il.reshape(-1, 16).T, (8, 1)).copy()
        xT_c = np.zeros((SIT + 1, T_pad), np.float32)
        xT_c[:SIT] = pc['x_c'].T
        xT_c[SIT] = pc['bias_c']
        # fresh stream: wave-1 chunks, zeros for pads; shipped bf16 both in
        # natural [step, 2S] layout and pre-transposed [state, step] layout
        # (the transposed copy feeds the PE stationary directly)
        fresh_c = np.zeros((128, fresh_cols, S), np.float32)
        for (cs, nb, w, af, kc, fo) in blocks:
            if fo < 0:
                continue
            for q in range(nb):
                sl = slice((cs + q) * CHUNK, (cs + q + 1) * CHUNK)
                vb = pc['ob'][sl] >= 0
                vp = pc['op'][sl] >= 0
                fresh_c[vb, fo + 2 * q, :] = table0[pc['ob'][sl][vb]]
                fresh_c[vp, fo + 2 * q + 1, :] = table0[pc['op'][sl][vp]]
        fresh_b16 = fresh_c.astype(bf16)
        k1 = wave_chunks[0]
        freshT = np.zeros((128, k1, 4, CHUNK), bf16)
        for q in range(k1):
            hcat = np.concatenate([fresh_b16[:, 2 * q, :],
                                   fresh_b16[:, 2 * q + 1, :]], axis=1)
            for k in range(4):
                freshT[:, q, k, :] = hcat[:, CHUNK * k:CHUNK * (k + 1)].T
        # wave-2 H tile pre-fill: table0 value of every referenced row
        # (scattered dh accumulates on top to form the post-wave-1 value)
        k2 = wave_chunks[1]
        fresh2 = np.zeros((128, 2 * k2, S), np.float32)
        for q in range(g0, T_pad):
            if st[q] < 0:
                continue
            cc = q // CHUNK - wave_chunks[0]
            j = q % CHUNK
            fresh2[j, 2 * cc, :] = table0[ob[q]]
            fresh2[j, 2 * cc + 1, :] = table0[op[q]]
        dmask_c = pc['dm_c'].reshape(n_chunks, CHUNK).T.copy()
        per_core[c] = dict(idx_rep=idx_rep, xT=xT_c.astype(bf16),
                           fresh_arr=fresh_b16, freshT=freshT,
                           fresh2=fresh2.astype(bf16), dmask=dmask_c,
                           ob=ob, op=op)

    WzT = np.concatenate([Wz.T, bz[None, :]], axis=0)
    WrT = np.concatenate([Wr.T, -br[None, :]], axis=0)
    WhT = np.concatenate([Wh.T, bh[None, :]], axis=0)

    def ut(U):
        return np.ascontiguousarray(U.T.reshape(4, 128, S2).transpose(1, 0, 2))

    hd = dict(
        WzT=WzT.astype(bf16), WrT=WrT.astype(bf16), WhT=WhT.astype(bf16),
        UzT=ut(Uz).astype(bf16), UrT=ut(Ur).astype(bf16),
        UhT=ut(Uh).astype(bf16),
        n_chunks=n_chunks, blocks=blocks, fresh_cols=fresh_cols,
        kc_wave=kc_wave, wave_chunks=wave_chunks, chunk_wave=chunk_wave,
        T_pad=T_pad, dup_any=dup_any,
        per_core=per_core,
        host_steps=sch['host_steps'], lev=sch['lev'],
        x=x, b=b, p=p, Wz=Wz, Wr=Wr, Wh=Wh, Uz=Uz, Ur=Ur, Uh=Uh,
        bz=bz, br=br, bh=bh,
    )
    return hd


def _build_nc(hd):
    import concourse.bacc as bacc
    import concourse.mybir as mybir
    import concourse.tile as tile
    from concourse.masks import make_identity

    n_chunks = hd['n_chunks']
    T_pad = hd['T_pad']
    blocks = hd['blocks']
    f32 = mybir.dt.float32
    bf16 = mybir.dt.bfloat16
    i16 = mybir.dt.int16

    nc = bacc.Bacc("TRN2", target_bir_lowering=False, debug=True)

    idx_in = nc.dram_tensor("idx", (128, 2 * T_pad // 16), i16, kind="ExternalInput")
    fresh_in = nc.dram_tensor("fresh", (128, hd['fresh_cols'], S), bf16,
                              kind="ExternalInput")
    fresh2_in = nc.dram_tensor("fresh2", (128, 2 * hd['wave_chunks'][1], S),
                               bf16, kind="ExternalInput")
    freshT_in = nc.dram_tensor("freshT", (128, hd['wave_chunks'][0], 4, CHUNK),
                               bf16, kind="ExternalInput")
    xT_in = nc.dram_tensor("xT", (SIT + 1, T_pad), bf16, kind="ExternalInput")
    WzT_in = nc.dram_tensor("WzT", (SIT + 1, S2), bf16, kind="ExternalInput")
    WrT_in = nc.dram_tensor("WrT", (SIT + 1, S2), bf16, kind="ExternalInput")
    WhT_in = nc.dram_tensor("WhT", (SIT + 1, S2), bf16, kind="ExternalInput")
    UzT_in = nc.dram_tensor("UzT", (128, 4, S2), bf16, kind="ExternalInput")
    UrT_in = nc.dram_tensor("UrT", (128, 4, S2), bf16, kind="ExternalInput")
    UhT_in = nc.dram_tensor("UhT", (128, 4, S2), bf16, kind="ExternalInput")
    dmask_in = nc.dram_tensor("dmask", (128, n_chunks), f32, kind="ExternalInput")

    dh_out = nc.dram_tensor("dh", (128, 2 * n_chunks, S), bf16,
                            kind="ExternalOutput")

    Sig = mybir.ActivationFunctionType.Sigmoid
    Tanh = mybir.ActivationFunctionType.Tanh

    wave_chunks = hd['wave_chunks']
    kc_wave = hd['kc_wave']
    chunk_wave = hd['chunk_wave']

    with tile.TileContext(nc) as tc:
        with tc.tile_pool(name="const", bufs=1) as cpool, \
             tc.tile_pool(name="gath", bufs=8) as gpool, \
             tc.tile_pool(name="dhb", bufs=4) as dhpool, \
             tc.tile_pool(name="work", bufs=4) as wpool, \
             tc.tile_pool(name="psA", bufs=2, space="PSUM") as psA, \
             tc.tile_pool(name="psZ", bufs=2, space="PSUM") as psZ, \
             tc.tile_pool(name="psR", bufs=2, space="PSUM") as psR, \
             tc.tile_pool(name="psM", bufs=2, space="PSUM") as psM:

            # ---- static loads (sync HWDGE), most-urgent first ----
            xT_sb = cpool.tile([SIT + 1, T_pad], bf16, tag="xT")
            nc.sync.dma_start(xT_sb[:], xT_in[:])
            w_sb = {}
            for nm, t in (("WzT", WzT_in), ("WrT", WrT_in)):
                w_sb[nm] = cpool.tile([SIT + 1, S2], bf16, tag=nm, name=nm + "_sb")
                nc.sync.dma_start(w_sb[nm][:], t[:])
            for nm, t in (("UzT", UzT_in), ("UrT", UrT_in)):
                w_sb[nm] = cpool.tile([128, 4, S2], bf16, tag=nm, name=nm + "_sb")
                nc.sync.dma_start(w_sb[nm][:], t[:])
            gtiles = {}
            httiles = {}

            def emit_fresh(c):
                g = gpool.tile([128, 2, S], bf16, tag="hg", name=f"hg_{c}")
                nc.sync.dma_start(g[:], fresh_in[:, 2 * c:2 * c + 2, :])
                ht = gpool.tile([128, 4, CHUNK], bf16, tag="hT", name=f"hT_{c}")
                nc.sync.dma_start(ht[:], freshT_in[:, c, :, :])
                return g, ht

            # first chunk's rows before the remaining weights
            gtiles[0], httiles[0] = emit_fresh(0)
            w_sb["WhT"] = cpool.tile([SIT + 1, S2], bf16, tag="WhT", name="WhT_sb")
            nc.sync.dma_start(w_sb["WhT"][:], WhT_in[:])
            w_sb["UhT"] = cpool.tile([128, 4, S2], bf16, tag="UhT", name="UhT_sb")
            nc.sync.dma_start(w_sb["UhT"][:], UhT_in[:])
            # wave-2 H tile: host pre-fill (table0 rows); wave-1 scatters
            # accumulate dh directly into it (parity-split SBUF scatter)
            k2 = wave_chunks[1]
            cons = cpool.tile([128, 2 * k2, S], bf16, tag="cons")
            nc.sync.dma_start(cons[:], fresh2_in[:])
            trash = cpool.tile([128, 2 * k2, S], bf16, tag="trash")
            nc.scalar.memzero(trash[:])
            for c in range(1, wave_chunks[0]):
                gtiles[c], httiles[c] = emit_fresh(c)
            idx_sb = cpool.tile([128, 2 * T_pad // 16], i16, tag="idx")
            nc.sync.dma_start(idx_sb[:], idx_in[:])
            dmask_sb = cpool.tile([128, n_chunks], f32, tag="dmask")
            if hd['dup_any']:
                nc.sync.dma_start(dmask_sb[:], dmask_in[:])
            identb = cpool.tile([128, 128], bf16, tag="identb")
            make_identity(nc, identb[:])
            # preload the Q7 SWDGE library now; the lazy auto-load would sit
            # right before the first scatter, on the critical path
            from concourse import library_config
            nc.gpsimd.load_library(library_config.mlp)

            for c in range(n_chunks):
                w = int(chunk_wave[c])
                if w == 0:
                    g = gtiles.pop(c)[:]
                else:
                    cw = c - wave_chunks[0]
                    g = cons[:, 2 * cw:2 * cw + 2, :]
                hg2 = g.rearrange("p a b -> p (a b)")
                dhb = dhpool.tile([128, 2, S], bf16, tag="dh", name=f"dh_{c}")

                tr_ps_f = psA.tile([128, 4, CHUNK], f32, tag="tr",
                                   name=f"trp_{c}")
                tr_ps = tr_ps_f[:].bitcast(bf16)
                if c in httiles:
                    ht = httiles.pop(c)      # host-packed transposed stream
                else:
                    # PE transpose of the gathered (bf16) rows
                    ht_ps = tr_ps[:, :, 0:CHUNK]
                    for k in range(4):
                        nc.tensor.transpose(
                            ht_ps[:, k, :], hg2[:, CHUNK * k:CHUNK * (k + 1)],
                            identb[:])
                    ht = wpool.tile([128, 4, CHUNK], bf16, tag="ht")
                    nc.vector.tensor_copy(ht[:], ht_ps)

                xt_c = xT_sb[:, CHUNK * c:CHUNK * (c + 1)]

                zpre = psZ.tile([128, S2], f32, tag="zpre")
                rpre = psR.tile([128, S2], f32, tag="rpre")
                nc.tensor.matmul(zpre[:], xt_c, w_sb["WzT"][:],
                                 start=True, stop=False)
                nc.tensor.matmul(rpre[:], xt_c, w_sb["WrT"][:],
                                 start=True, stop=False)
                for k in range(4):
                    nc.tensor.matmul(zpre[:], ht[:, k, :], w_sb["UzT"][:, k, :],
                                     start=False, stop=(k == 3))
                    nc.tensor.matmul(rpre[:], ht[:, k, :], w_sb["UrT"][:, k, :],
                                     start=False, stop=(k == 3))

                zc = wpool.tile([128, S2], f32, tag="zc")
                r = wpool.tile([128, S2], f32, tag="r")
                nc.scalar.activation(zc[:], zpre[:], Sig, scale=-1.0)  # 1-z
                nc.scalar.activation(r[:], rpre[:], Sig)

                rh = wpool.tile([128, S2], bf16, tag="rh")
                nc.vector.tensor_mul(rh[:], r[:], hg2)
                rht_ps = tr_ps[:, :, CHUNK:2 * CHUNK]
                for k in range(4):
                    nc.tensor.transpose(
                        rht_ps[:, k, :], rh[:, CHUNK * k:CHUNK * (k + 1)],
                        identb[:])
                rht = wpool.tile([128, 4, CHUNK], bf16, tag="rht")
                nc.vector.tensor_copy(rht[:], rht_ps)

                mpre = psM.tile([128, S2], f32, tag="mpre")
                nc.tensor.matmul(mpre[:], xt_c, w_sb["WhT"][:],
                                 start=True, stop=False)
                for k in range(4):
                    nc.tensor.matmul(mpre[:], rht[:, k, :], w_sb["UhT"][:, k, :],
                                     start=False, stop=(k == 3))

                m = wpool.tile([128, S2], f32, tag="m")
                nc.scalar.activation(m[:], mpre[:], Tanh)

                # dh = (1-z)*(m-h)
                t1 = wpool.tile([128, S2], f32, tag="t1")
                nc.vector.tensor_sub(t1[:], m[:], hg2)
                dh_view = dhb[:].rearrange("p a b -> p (a b)")
                nc.vector.tensor_mul(dh_view, zc[:], t1[:])
                if hd['dup_any']:
                    tm = wpool.tile([128, S], bf16, tag="tm")
                    nc.vector.tensor_scalar_mul(
                        tm[:], dhb[:, 1, :], dmask_sb[:, c:c + 1])
                    nc.vector.tensor_add(
                        dhb[:, 0, :], dhb[:, 0, :], tm[:])

                # ship deltas to host (sync HWDGE)
                nc.sync.dma_start(dh_out[:, 2 * c:2 * c + 2, :], dhb[:])
                cw0 = c - int(np.sum(wave_chunks[:w]))
                if w < DW - 1 and cw0 < kc_wave[w]:
                    nc.gpsimd.dma_scatter_add(
                        cons[:], dhb[:],
                        idx_sb[:, 16 * c:16 * (c + 1)],
                        2 * CHUNK, 2 * CHUNK, S, queue_num=0,
                        sbuf_tokens_per_rank=128, parity_reg=0,
                        out_ap_other=trash[:],
                    )

    nc.compile()
    return nc


def _in_map(hd, core):
    pc = hd['per_core'][core]
    return {
        "idx": pc['idx_rep'], "fresh": pc['fresh_arr'],
        "freshT": pc['freshT'], "fresh2": pc['fresh2'], "xT": pc['xT'],
        "WzT": hd['WzT'], "WrT": hd['WrT'], "WhT": hd['WhT'],
        "UzT": hd['UzT'], "UrT": hd['UrT'], "UhT": hd['UhT'],
        "dmask": pc['dmask'],
    }


def _run(hd, nc, trace=False):
    from concourse.bass_utils import run_bass_kernel_spmd
    return run_bass_kernel_spmd(nc, [_in_map(hd, c) for c in range(8)],
                                list(range(8)), trace=trace)


def _assemble(hd, dh_cores, table0):
    """Apply device deltas (rows never cross cores), then finish the tail
    waves on host (same-level steps never share a row -> batched GEMMs)."""
    n_chunks = hd['n_chunks']
    out = table0.astype(np.float32).copy()
    for cidx in range(8):
        dh = np.ascontiguousarray(dh_cores[cidx].transpose(1, 0, 2))
        dh = dh.reshape(n_chunks, 2, CHUNK, S).transpose(0, 2, 1, 3)
        dh = dh.reshape(hd['T_pad'] * 2, S)
        pc = hd['per_core'][cidx]
        rows = np.stack([pc['ob'], pc['op']], axis=1).reshape(-1)
        valid = rows >= 0
        np.add.at(out, rows[valid], dh[valid])

    hs = np.asarray(hd['host_steps'], np.int64)
    if len(hs):
        x, b, p = hd['x'], hd['b'], hd['p']
        Wz, Wr, Wh = hd['Wz'], hd['Wr'], hd['Wh']
        Uz, Ur, Uh = hd['Uz'], hd['Ur'], hd['Uh']
        bz, br, bh = hd['bz'], hd['br'], hd['bh']
        levs = hd['lev'][hs]
        for L in np.unique(levs):
            ts = hs[levs == L]
            H = np.concatenate([out[b[ts]], out[p[ts]]], axis=1)
            Z = 1 / (1 + np.exp(-(x[ts] @ Wz.T + H @ Uz.T + bz)))
            R = 1 / (1 + np.exp(-(x[ts] @ Wr.T + H @ Ur.T - br)))
            M = np.tanh(x[ts] @ Wh.T + (R * H) @ Uh.T + bh)
            dh = (1.0 - Z) * (M - H)
            np.add.at(out, b[ts], dh[:, :S])
            np.add.at(out, p[ts], dh[:, S:])
    return out


def kernel(**inputs):
    x = np.asarray(inputs['x'], dtype=np.float32)
    b = np.asarray(inputs['b'])
    p = np.asarray(inputs['p'])
    table0 = np.asarray(inputs['table0'], dtype=np.float32)

    hd = _build_host_data(
        x, b, p,
        np.asarray(inputs['Wz'], np.float32), np.asarray(inputs['Wr'], np.float32),
        np.asarray(inputs['Wh'], np.float32), np.asarray(inputs['Uz'], np.float32),
        np.asarray(inputs['Ur'], np.float32), np.asarray(inputs['Uh'], np.float32),
        np.asarray(inputs['bz'], np.float32), np.asarray(inputs['br'], np.float32),
        np.asarray(inputs['bh'], np.float32), table0)

    nc = _build_nc(hd)
    res = _run(hd, nc)
    dh_cores = [np.asarray(res.results[c]["dh"], np.float32) for c in range(8)]
    return _assemble(hd, dh_cores, table0)


if __name__ == "__main__":
    d = np.load('/tmp/ref_inputs.npz')
    inputs = {k: d[k] for k in d.files}
    got = kernel(**inputs)
    exp = np.load('/tmp/ref_out_np.npy')
    err = np.abs(got - exp).max()
    print("abs err:", err, "rel:", err / np.abs(exp).max())
